# revision 19
# baseline (speedup 1.0000x reference)
"""CNN + truncated path-signature (depth 4) + FF head on 8 TRN2 NeuronCores.

Strategy
--------
- Batch data-parallel signature computation: core c handles batches
  [8c, 8c+8) = 32 (batch, out_ch) lanes, T=128 time steps on partitions.
- Signature reformulated to avoid sequential scans:
    dx, P1 (= shifted path), Y (= suffix sums) are free;
    the only prefix scan (level 2, S2pre) is one triangular matmul L @ M2;
    S3 = B^T X;  S4 = B^T R + tmp81'^T (dx(x)dx)/2   (suffix-vector trick),
  so levels 3 and 4 are plain T-contractions on the TensorEngine.
- The 60 MB w0 is row-sharded 8 ways (this is the memory roofline win):
  AllToAll re-shards the signature activations feature-wise, each core
  multiplies its [3690, 512] w0 shard, ReduceScatter returns each core its
  own 8 batches, and the small w1/w2 head finishes per-core.
- w0 rows are permuted host-side to match the kernel's natural feature
  order, so no on-device transposes are needed.
"""
import os
import sys
sys.path.insert(0, "/opt/trn_rl_repo")
if os.environ.get("JAX_PLATFORMS") == "cpu":
    # The SPMD launch needs the axon/neuron PJRT backend.
    os.environ["JAX_PLATFORMS"] = ""

import numpy as np
import bass_rust as _bass_rust
import concourse.bass as bass
import concourse.tile as tile
import concourse.mybir as mybir
from concourse.vector_clock import ScopedClock
from concourse.bass_utils import run_bass_kernel_spmd

F32 = mybir.dt.float32
F32R = mybir.dt.float32r
AL = mybir.AluOpType
AF = mybir.ActivationFunctionType

NCORES = 8
B, T, IN_CH = 64, 128, 32
OUT_CH, CH, D = 4, 8, 9            # conv out-channels, conv width, path dim
BL = B // NCORES                   # local batches = 8
LANES = BL * OUT_CH                # 32 lanes/core
NG = 4                             # lane groups of 8
GL = 8                             # lanes per group
SIGC = 7380                        # per-lane signature channels
KSH = OUT_CH * SIGC // NCORES      # w0 K-shard rows per core = 3690
H0, H1, NCLS = 512, 256, 10


class _SplitDrainTileContext(tile.TileContext):
    """Tile exit drain carries one sem wait per CTRL instruction.

    This container's walrus build rejects >2 sync waits on a CTRL
    instruction; Tile's exit drain waits on the whole global clock.
    Redistribute the waits over nops on the same engine (program order on
    one engine preserves semantics)."""

    MAX_WAITS = 1

    def _split_body_waits(self):
        """Move excess sem waits from any instruction onto preceding nops on
        the same engine (same-engine program order preserves semantics)."""
        nc = self.nc
        for bb in nc.main_func.blocks:
            heavy = [ins for ins in bb.instructions
                     if ins.sync_info and ins.sync_info.on_wait
                     and len(ins.sync_info.on_wait) > self.MAX_WAITS]
            if not heavy:
                continue
            extra = {}
            for ins in heavy:
                w = list(ins.sync_info.on_wait)
                ins.sync_info.on_wait = w[:self.MAX_WAITS]
                nops = []
                for i in range(self.MAX_WAITS, len(w), self.MAX_WAITS):
                    n = nc.engines[ins.engine].nop(hint="wait_split")
                    # pop the freshly appended nop from wherever it landed
                    for bb2 in nc.main_func.blocks:
                        if bb2.instructions and bb2.instructions[-1] is n.ins:
                            bb2.instructions.pop()
                            break
                    for wt in w[i:i + self.MAX_WAITS]:
                        handle = _bass_rust.SemaphoreHandle(wt.ant_name, wt.id)
                        _bass_rust.wait_op(n.ins, handle, wt.wait_value,
                                           "sem-ge", False)
                    nops.append(n.ins)
                extra[id(ins)] = nops
            new_list = []
            for ins in bb.instructions:
                new_list.extend(extra.get(id(ins), ()))
                new_list.append(ins)
            bb.instructions[:] = new_list

    def _drain_and_barrier(self, tick_clock, wait_clock):
        nc = self.nc
        self._split_body_waits()
        probe = nc.sync.nop(hint="tile_exit_wait_0")
        wait_clock.add_sem_waits(
            probe.ins, ScopedClock({None: tick_clock.global_clock})
        )
        waits = list(probe.ins.sync_info.on_wait or [])
        probe.ins.sync_info.on_wait = waits[:1]
        for w in waits[1:]:
            n = nc.sync.nop(hint="tile_exit_wait")
            handle = _bass_rust.SemaphoreHandle(w.ant_name, w.id)
            _bass_rust.wait_op(n.ins, handle, w.wait_value, "sem-ge", False)
        nc.sync.drain()
        nc.all_engine_barrier()
        assert self.sems is not None
        popped = nc._tile_sem_poison_stack.pop()
        assert popped is self._sem_poison
        nc.clear_and_free_semaphores(list(self.sems.allocated().values()))
        nc.all_engine_barrier()


def _ap(t, extra, *dims):
    """AP over tile t's buffer: partition dim from the tile, custom free dims.

    dims[0] may override the partition [step, count]."""
    base = t[:]
    return bass.AP(base.tensor, base.offset + extra, list(dims))


def _w0_perm():
    """Row permutation p s.t. w0_permuted[i] = w0[p[i]] matches the kernel's
    feature order: per oc: [S1(9) | S2(81)] then rows 90 + ij*90 + c with
    c<81 -> level4 (ij,kl=c), c>=81 -> level3 (ij, k=c-81)."""
    p = np.empty(OUT_CH * SIGC, np.int64)
    i = 0
    for oc in range(OUT_CH):
        base = oc * SIGC
        p[i:i + 90] = base + np.arange(90)          # S1 then S2, native order
        i += 90
        for ij in range(81):
            # level-4 block (ij, kl) for kl in 0..80 -> orig 819 + ij*81 + kl
            p[i:i + 81] = base + 819 + ij * 81 + np.arange(81)
            i += 81
            # level-3 (ij, k) for k in 0..8 -> orig 90 + ij*9 + k
            p[i:i + 9] = base + 90 + ij * 9 + np.arange(9)
            i += 9
    assert i == OUT_CH * SIGC
    return p


def _build():
    nc = bass.Bass(num_devices=NCORES, target_bir_lowering=True, trn_type="TRN2")

    # ---- per-core DRAM inputs ----
    xs = nc.dram_tensor("xs", [T, BL, IN_CH], F32, kind="ExternalInput")
    cwr = nc.dram_tensor("cwr", [128, 16], F32, kind="ExternalInput")
    cbr = nc.dram_tensor("cbr", [128, OUT_CH], F32, kind="ExternalInput")
    tlin = nc.dram_tensor("tlin", [128, 1], F32, kind="ExternalInput")
    ltri = nc.dram_tensor("ltri", [128, 128], F32R, kind="ExternalInput")
    onef = nc.dram_tensor("onef", [128, 1], F32, kind="ExternalInput")
    oner = nc.dram_tensor("oner", [128, 1], F32R, kind="ExternalInput")
    idn = nc.dram_tensor("idn", [128, 128], F32, kind="ExternalInput")
    w0s = nc.dram_tensor("w0s", [KSH, H0], F32R, kind="ExternalInput")
    w1s = nc.dram_tensor("w1s", [H0, H1], F32, kind="ExternalInput")
    w2s = nc.dram_tensor("w2s", [H1, NCLS], F32, kind="ExternalInput")
    b0c = nc.dram_tensor("b0c", [H0, 1], F32, kind="ExternalInput")
    b1c = nc.dram_tensor("b1c", [H1, 1], F32, kind="ExternalInput")
    b2r = nc.dram_tensor("b2r", [BL, NCLS], F32, kind="ExternalInput")
    out = nc.dram_tensor("out", [BL, NCLS], F32, kind="ExternalOutput")

    with _SplitDrainTileContext(nc) as tc:
        with tc.tile_pool(name="dram", bufs=1, space="DRAM") as dram:
            zl = dram.tile([NCORES * KSH, BL], F32R)     # local z^T, f'-order
            zex = dram.tile([NCORES * KSH, BL], F32R)    # post-A2A shard
            cin = dram.tile([B, H0], F32)
            cout = dram.tile([BL, H0], F32)
            prow = dram.tile([1, LANES * D], F32)        # p[T-1] bounce

            with tc.tile_pool(name="const", bufs=1) as cpool, \
                 tc.tile_pool(name="w0p", bufs=1) as w0pool, \
                 tc.tile_pool(name="prep", bufs=1) as ppool:
                lt_sb = cpool.tile([128, 128], F32R)
                nc.sync.dma_start(lt_sb[:], ltri[:])
                onef_sb = cpool.tile([128, 1], F32)
                nc.sync.dma_start(onef_sb[:], onef[:])
                oner_sb = cpool.tile([128, 1], F32R)
                nc.sync.dma_start(oner_sb[:], oner[:])
                tl_sb = cpool.tile([128, 1], F32)
                nc.sync.dma_start(tl_sb[:], tlin[:])
                cw_sb = cpool.tile([128, 16], F32)
                nc.sync.dma_start(cw_sb[:], cwr[:])
                cb_sb = cpool.tile([128, OUT_CH], F32)
                nc.sync.dma_start(cb_sb[:], cbr[:])
                idn_sb = cpool.tile([128, 128], F32)
                nc.sync.dma_start(idn_sb[:], idn[:])
                w1_sb = [cpool.tile([128, H1], F32, tag=f"w1_{j}", name=f"w1_{j}")
                         for j in range(4)]
                for j in range(4):
                    nc.sync.dma_start(w1_sb[j][:], w1s[j * 128:(j + 1) * 128, :])
                w2_sb = [cpool.tile([128, NCLS], F32, tag=f"w2_{j}", name=f"w2_{j}")
                         for j in range(2)]
                for j in range(2):
                    nc.sync.dma_start(w2_sb[j][:], w2s[j * 128:(j + 1) * 128, :])
                b0_sb = cpool.tile([128, 4], F32)
                nc.sync.dma_start(b0_sb[:], _ap(b0c, 0, [1, 128], [128, 4]))
                b1_sb = cpool.tile([128, 2], F32)
                nc.sync.dma_start(b1_sb[:], _ap(b1c, 0, [1, 128], [128, 2]))
                b2_sb = cpool.tile([BL, NCLS], F32)
                nc.sync.dma_start(b2_sb[:], b2r[:])
                xs_sb = cpool.tile([128, BL * IN_CH], F32)
                nc.sync.dma_start(xs_sb[:], _ap(xs, 0, [BL * IN_CH, 128],
                                                [1, BL * IN_CH]))
                # prefetch the whole w0 shard (28x128 + 106 rows)
                w0_t = []
                for i in range(29):
                    k0 = i * 128
                    kn = min(128, KSH - k0)
                    wt = w0pool.tile([128, H0], F32R, tag=f"w0_{i}", name=f"w0_{i}")
                    nc.sync.dma_start(wt[0:kn, :], w0s[k0:k0 + kn, :])
                    w0_t.append((wt, k0, kn))

                # ---- prep: conv -> path p, then dx, P1, Y, u, u2, A ----
                W = LANES * D  # 288
                p = ppool.tile([128, W], F32, tag="p")
                dx = ppool.tile([128, W], F32, tag="dx")
                p1 = ppool.tile([128, W], F32, tag="p1")
                yt = ppool.tile([128, W], F32, tag="yt")
                pl = ppool.tile([128, W], F32, tag="pl")
                ut = ppool.tile([128, W], F32, tag="ut")
                u2 = ppool.tile([128, W], F32, tag="u2")
                at = ppool.tile([128, W], F32, tag="at")

                for oc in range(OUT_CH):
                    dst = _ap(p, oc * D + 1, [W, 128], [4 * D, BL], [1, CH])
                    for k in range(4):
                        src = _ap(xs_sb, k, [BL * IN_CH, 128], [IN_CH, BL], [4, CH])
                        wsc = cw_sb[:, oc * 4 + k:oc * 4 + k + 1]
                        if k == 0:
                            nc.vector.tensor_scalar(dst, src, wsc, None, AL.mult)
                        else:
                            nc.vector.scalar_tensor_tensor(
                                dst, src, wsc, dst, AL.mult, AL.add)
                    nc.vector.tensor_scalar(dst, dst,
                                            cb_sb[:, oc:oc + 1], None, AL.add)
                # time channel into col 0 of every lane
                nc.vector.tensor_copy(_ap(p, 0, [W, 128], [D, LANES]),
                                      _ap(tl_sb, 0, [1, 128], [0, LANES]))
                # P1 = p shifted down one step (DMA: compute engines cannot
                # address unaligned partition bases), then dx = p - P1.
                nc.gpsimd.memset(p1[0:1, :], 0.0)
                nc.sync.dma_start(p1[1:128, :], p[0:127, :])
                nc.vector.tensor_tensor(dx[:], p[:], p1[:], AL.subtract)
                # Y[t] = p[T-1] - p[t]  (broadcast last row via DRAM bounce)
                nc.sync.dma_start(prow[:], p[127:128, :])
                nc.sync.dma_start(pl[:], _ap(prow, 0, [0, 128], [1, W]))
                nc.vector.tensor_tensor(yt[:], pl[:], p[:], AL.subtract)
                # ut = P1/2 + dx/6, u2 = P1/3 + dx/12, A = P1 + dx/2,
                # dxh = dx/2 (scalars pre-folded so the group outer products
                # are plain TensorTensor ops)
                nc.vector.tensor_scalar(ut[:], p1[:], 0.5, None, AL.mult)
                nc.vector.scalar_tensor_tensor(ut[:], dx[:], 1.0 / 6, ut[:],
                                               AL.mult, AL.add)
                nc.vector.tensor_scalar(u2[:], p1[:], 1.0 / 3, None, AL.mult)
                nc.vector.scalar_tensor_tensor(u2[:], dx[:], 1.0 / 12, u2[:],
                                               AL.mult, AL.add)
                nc.vector.scalar_tensor_tensor(at[:], dx[:], 0.5, p1[:],
                                               AL.mult, AL.add)
                dxh = ppool.tile([128, W], F32, tag="dxh")
                nc.vector.tensor_scalar(dxh[:], dx[:], 0.5, None, AL.mult)

                # ---- per-lane feature tiles, filled group by group ----
                zt4 = [cpool.tile([81, 720], F32R, tag=f"zt4_{oc}", name=f"zt4_{oc}")
                       for oc in range(OUT_CH)]
                z1f = [cpool.tile([9, BL], F32R, tag=f"z1f_{oc}", name=f"z1f_{oc}")
                       for oc in range(OUT_CH)]
                z2f = [cpool.tile([81, BL], F32R, tag=f"z2f_{oc}", name=f"z2f_{oc}")
                       for oc in range(OUT_CH)]

                with tc.tile_pool(name="grp", bufs=2) as gpool, \
                     tc.tile_pool(name="ps2", bufs=2, space="PSUM") as ps2, \
                     tc.tile_pool(name="ptab", bufs=1, space="PSUM") as ptab, \
                     tc.tile_pool(name="ps12", bufs=1, space="PSUM") as ps12:
                    for g in range(NG):
                        off = g * GL * D  # col offset into the 288-wide tiles
                        GW = GL * 81      # 648

                        def o_ij(t, st=1):  # [lane, i(step), j(bcast)] view
                            return _ap(t, off, [W, 128], [D, GL], [st, D], [0, D])

                        def o_ji(t, st=1):  # [lane, i(bcast), j(step)] view
                            return _ap(t, off, [W, 128], [D, GL], [0, D], [st, D])

                        m2 = gpool.tile([128, GW], F32R, tag="m2")
                        m2v = _ap(m2, 0, [GW, 128], [81, GL], [D, D], [1, D])
                        nc.vector.tensor_tensor(m2v, o_ij(at), o_ji(dx), AL.mult)
                        # fp32 alias for the N=1 column-sum matmul (f32r
                        # matmuls reject N=1 in this walrus build)
                        m2f = gpool.tile([128, GW], F32, tag="m2f")
                        nc.vector.tensor_copy(m2f[:], m2[:])

                        s2 = ps2.tile([128, GW], F32, tag="s2")
                        nc.tensor.matmul(s2[:, 0:512], lt_sb[:], m2[:, 0:512],
                                         start=True, stop=True)
                        nc.tensor.matmul(s2[:, 512:GW], lt_sb[:], m2[:, 512:GW],
                                         start=True, stop=True)

                        bt = gpool.tile([128, GW], F32, tag="bt")
                        btv = _ap(bt, 0, [GW, 128], [81, GL], [D, D], [1, D])
                        nc.vector.tensor_tensor(btv, o_ij(ut), o_ji(dx), AL.mult)
                        nc.vector.tensor_tensor(bt[:], bt[:], s2[:], AL.add)

                        t8 = gpool.tile([128, GW], F32, tag="t8")
                        t8v = _ap(t8, 0, [GW, 128], [81, GL], [D, D], [1, D])
                        nc.vector.tensor_tensor(t8v, o_ij(u2), o_ji(dx), AL.mult)
                        nc.vector.tensor_tensor(t8[:], t8[:], s2[:], AL.add)

                        q2 = gpool.tile([128, GW], F32, tag="q2")
                        q2v = _ap(q2, 0, [GW, 128], [81, GL], [D, D], [1, D])
                        nc.vector.tensor_tensor(q2v, o_ij(dxh), o_ji(dx), AL.mult)

                        rx = gpool.tile([128, GL * 90], F32, tag="rx")
                        rxv = _ap(rx, 0, [GL * 90, 128], [90, GL], [D, D], [1, D])
                        nc.vector.tensor_tensor(rxv, o_ij(dx), o_ji(yt), AL.mult)
                        nc.vector.tensor_copy(
                            _ap(rx, 81, [GL * 90, 128], [90, GL], [1, D]),
                            _ap(dx, off, [W, 128], [D, GL], [1, D]))

                        tab = ptab.tile([128, 1024], F32, tag="tab")
                        s12 = ps12.tile([128, 2 * GL], F32, tag="s12")
                        for l in range(GL):
                            lane = g * GL + l
                            bloc, oc = lane // OUT_CH, lane % OUT_CH
                            nc.tensor.matmul(
                                _ap(tab, 128 * l, [1024, 81], [1, 90]),
                                bt[:, l * 81:(l + 1) * 81],
                                rx[:, l * 90:(l + 1) * 90],
                                start=True, stop=False)
                            nc.tensor.matmul(
                                _ap(tab, 128 * l, [1024, 81], [1, 81]),
                                t8[:, l * 81:(l + 1) * 81],
                                q2[:, l * 81:(l + 1) * 81],
                                start=False, stop=True)
                            nc.tensor.matmul(s12[0:9, 2 * l:2 * l + 1],
                                             dx[:, lane * D:(lane + 1) * D],
                                             onef_sb[:], start=True, stop=True)
                            nc.tensor.matmul(s12[0:81, 2 * l + 1:2 * l + 2],
                                             m2f[:, l * 81:(l + 1) * 81],
                                             onef_sb[:], start=True, stop=True)
                        for l in range(GL):
                            lane = g * GL + l
                            bloc, oc = lane // OUT_CH, lane % OUT_CH
                            nc.scalar.activation(
                                _ap(zt4[oc], bloc, [720, 81], [8, 90]),
                                _ap(tab, 128 * l, [1024, 81], [1, 90]),
                                AF.Copy)
                            nc.scalar.activation(
                                z1f[oc][:, bloc:bloc + 1],
                                s12[0:9, 2 * l:2 * l + 1], AF.Copy)
                            nc.scalar.activation(
                                z2f[oc][:, bloc:bloc + 1],
                                s12[0:81, 2 * l + 1:2 * l + 2], AF.Copy)

                # ---- pack z^T shard-ordered into DRAM and exchange ----
                for oc in range(OUT_CH):
                    base = oc * SIGC * BL
                    nc.sync.dma_start(
                        _ap(zl, base, [8, 9], [1, 8]), z1f[oc][:])
                    nc.sync.dma_start(
                        _ap(zl, base + 9 * BL, [8, 81], [1, 8]), z2f[oc][:])
                    nc.sync.dma_start(
                        _ap(zl, base + 90 * BL, [720, 81], [8, 90], [1, 8]),
                        zt4[oc][:])
                nc.gpsimd.collective_compute(
                    "AllToAll", AL.bypass,
                    replica_groups=[list(range(NCORES))],
                    ins=[zl[:].opt()], outs=[zex[:].opt()])

                # ---- z0 = z @ w0 partial over this core's K shard ----
                with tc.tile_pool(name="zex", bufs=4) as zpool, \
                     tc.tile_pool(name="pz0", bufs=1, space="PSUM") as pz0p, \
                     tc.tile_pool(name="ptail", bufs=1, space="PSUM") as ptail:
                    z0p = pz0p.tile([B, H0], F32, tag="z0p")
                    for i, (wt, k0, kn) in enumerate(w0_t):
                        ztl = zpool.tile([128, B], F32R, tag="ztl")
                        nc.sync.dma_start(
                            ztl[0:kn, :],
                            _ap(zex, k0 * BL, [BL, kn],
                                [KSH * BL, NCORES], [1, BL]))
                        nc.tensor.matmul(z0p[:], ztl[0:kn, :], wt[0:kn, :],
                                         start=(i == 0), stop=(i == 28))
                    z0sb = cpool.tile([B, H0], F32, tag="z0sb", name="z0sb")
                    nc.vector.tensor_copy(z0sb[:], z0p[:])
                    nc.sync.dma_start(cin[:], z0sb[:])
                    nc.gpsimd.collective_compute(
                        "ReduceScatter", AL.add,
                        replica_groups=[list(range(NCORES))],
                        ins=[cin[:].opt()], outs=[cout[:].opt()])

                    # ---- tail: sigmoid(w0 out) -> w1 -> sigmoid -> w2 ----
                    z1row = cpool.tile([BL, H0], F32, tag="z1row")
                    nc.sync.dma_start(z1row[:], cout[:])
                    pz1 = ptail.tile([128, 4 * BL], F32, tag="pz1")
                    z1t = cpool.tile([128, 4 * BL], F32, tag="z1t")
                    for j in range(4):
                        nc.tensor.transpose(pz1[:, j * BL:(j + 1) * BL],
                                            z1row[:, j * 128:(j + 1) * 128],
                                            idn_sb[0:BL, 0:BL])
                        nc.scalar.activation(z1t[:, j * BL:(j + 1) * BL],
                                             pz1[:, j * BL:(j + 1) * BL],
                                             AF.Sigmoid, bias=b0_sb[:, j:j + 1])
                    pz2 = ptail.tile([128, 2 * BL], F32, tag="pz2")
                    z2t = cpool.tile([128, 2 * BL], F32, tag="z2t")
                    for m in range(2):
                        for kj in range(4):
                            nc.tensor.matmul(
                                pz2[:, m * BL:(m + 1) * BL],
                                w1_sb[kj][:, m * 128:(m + 1) * 128],
                                z1t[:, kj * BL:(kj + 1) * BL],
                                start=(kj == 0), stop=(kj == 3))
                        nc.scalar.activation(z2t[:, m * BL:(m + 1) * BL],
                                             pz2[:, m * BL:(m + 1) * BL],
                                             AF.Sigmoid, bias=b1_sb[:, m:m + 1])
                    pz3 = ptail.tile([BL, NCLS], F32, tag="pz3")
                    for m in range(2):
                        nc.tensor.matmul(pz3[:], z2t[:, m * BL:(m + 1) * BL],
                                         w2_sb[m][:], start=(m == 0),
                                         stop=(m == 1))
                    z3 = cpool.tile([BL, NCLS], F32, tag="z3")
                    nc.vector.tensor_tensor(z3[:], pz3[:], b2_sb[:], AL.add)
                    mx = cpool.tile([BL, 1], F32, tag="mx")
                    nc.vector.tensor_reduce(mx[:], z3[:], mybir.AxisListType.X,
                                            AL.max)
                    tm = cpool.tile([BL, NCLS], F32, tag="tm")
                    nc.vector.tensor_scalar(tm[:], z3[:], mx[:, 0:1], None,
                                            AL.subtract)
                    ex = cpool.tile([BL, NCLS], F32, tag="ex")
                    se = cpool.tile([BL, 1], F32, tag="se")
                    nc.scalar.activation(ex[:], tm[:], AF.Exp, accum_out=se[:])
                    ls = cpool.tile([BL, 1], F32, tag="ls")
                    nc.scalar.activation(ls[:], se[:], AF.Ln)
                    osb = cpool.tile([BL, NCLS], F32, tag="osb")
                    nc.vector.tensor_scalar(osb[:], tm[:], ls[:, 0:1], None,
                                            AL.subtract)
                    nc.sync.dma_start(out[:], osb[:])
    return nc


_CACHE = {}


def kernel(x, conv_w, conv_b, w0, b0, w1, b1, w2, b2):
    x = np.ascontiguousarray(np.asarray(x, np.float32))
    conv_w = np.asarray(conv_w, np.float32)
    conv_b = np.asarray(conv_b, np.float32)
    w0 = np.asarray(w0, np.float32)
    w1 = np.ascontiguousarray(np.asarray(w1, np.float32))
    w2 = np.ascontiguousarray(np.asarray(w2, np.float32))
    b0 = np.asarray(b0, np.float32)
    b1 = np.asarray(b1, np.float32)
    b2 = np.asarray(b2, np.float32)

    if "nc" not in _CACHE:
        _CACHE["nc"] = _build()
        _CACHE["perm"] = _w0_perm()
    nc = _CACHE["nc"]
    w0p = np.ascontiguousarray(w0[_CACHE["perm"], :])

    shared = {
        "cwr": np.ascontiguousarray(
            np.broadcast_to(conv_w.reshape(1, 16), (128, 16))),
        "cbr": np.ascontiguousarray(
            np.broadcast_to(conv_b.reshape(1, OUT_CH), (128, OUT_CH))),
        "tlin": np.linspace(0.0, 1.0, T, dtype=np.float32).reshape(128, 1),
        "ltri": np.ascontiguousarray(
            np.triu(np.ones((128, 128), np.float32), 1)),
        "onef": np.ones((128, 1), np.float32),
        "oner": np.ones((128, 1), np.float32),
        "idn": np.eye(128, dtype=np.float32),
        "w1s": w1, "w2s": w2,
        "b0c": b0.reshape(H0, 1), "b1c": b1.reshape(H1, 1),
        "b2r": np.ascontiguousarray(np.broadcast_to(b2.reshape(1, NCLS),
                                                    (BL, NCLS))),
    }
    in_maps = []
    for c in range(NCORES):
        m = dict(shared)
        m["xs"] = np.ascontiguousarray(
            x[c * BL:(c + 1) * BL, 0].transpose(1, 0, 2))
        m["w0s"] = np.ascontiguousarray(w0p[c * KSH:(c + 1) * KSH, :])
        in_maps.append(m)

    _CACHE["in_maps"] = in_maps
    res = run_bass_kernel_spmd(nc, in_maps, core_ids=list(range(NCORES)))
    return np.concatenate([res.results[c]["out"] for c in range(NCORES)],
                          axis=0)


# revision 25
# speedup vs baseline: 1.2685x; 1.2685x over previous
"""CNN + truncated path-signature (depth 4) + FF head on 8 TRN2 NeuronCores.

Strategy
--------
- Batch data-parallel signature computation: core c handles batches
  [8c, 8c+8) = 32 (batch, out_ch) lanes, T=128 time steps on partitions.
- Signature reformulated to avoid sequential scans:
    dx, P1 (= shifted path), Y (= suffix sums) are free;
    the only prefix scan (level 2, S2pre) is one triangular matmul L @ M2;
    S3 = B^T X;  S4 = B^T R + tmp81'^T (dx(x)dx)/2   (suffix-vector trick),
  so levels 3 and 4 are plain T-contractions on the TensorEngine.
- The 60 MB w0 is row-sharded 8 ways (this is the memory roofline win):
  AllToAll re-shards the signature activations feature-wise, each core
  multiplies its [3690, 512] w0 shard, ReduceScatter returns each core its
  own 8 batches, and the small w1/w2 head finishes per-core.
- w0 rows are permuted host-side to match the kernel's natural feature
  order, so no on-device transposes are needed.
"""
import os
import sys
sys.path.insert(0, "/opt/trn_rl_repo")
if os.environ.get("JAX_PLATFORMS") == "cpu":
    # The SPMD launch needs the axon/neuron PJRT backend.
    os.environ["JAX_PLATFORMS"] = ""

import numpy as np
import bass_rust as _bass_rust
import concourse.bass as bass
import concourse.tile as tile
import concourse.mybir as mybir
from concourse.vector_clock import ScopedClock
from concourse.bass_utils import run_bass_kernel_spmd

F32 = mybir.dt.float32
F32R = mybir.dt.float32r
BF16 = mybir.dt.bfloat16
AL = mybir.AluOpType
AF = mybir.ActivationFunctionType

NCORES = 8
B, T, IN_CH = 64, 128, 32
OUT_CH, CH, D = 4, 8, 9            # conv out-channels, conv width, path dim
BL = B // NCORES                   # local batches = 8
LANES = BL * OUT_CH                # 32 lanes/core
NG = 4                             # lane groups of 8
GL = 8                             # lanes per group
SIGC = 7380                        # per-lane signature channels
KSH = OUT_CH * SIGC // NCORES      # w0 K-shard rows per core = 3690
H0, H1, NCLS = 512, 256, 10


class _SplitDrainTileContext(tile.TileContext):
    """Tile exit drain carries one sem wait per CTRL instruction.

    This container's walrus build rejects >2 sync waits on a CTRL
    instruction; Tile's exit drain waits on the whole global clock.
    Redistribute the waits over nops on the same engine (program order on
    one engine preserves semantics)."""

    MAX_WAITS = 1

    def _split_body_waits(self):
        """Move excess sem waits from any instruction onto preceding nops on
        the same engine (same-engine program order preserves semantics)."""
        nc = self.nc
        for bb in nc.main_func.blocks:
            heavy = [ins for ins in bb.instructions
                     if ins.sync_info and ins.sync_info.on_wait
                     and len(ins.sync_info.on_wait) > self.MAX_WAITS]
            if not heavy:
                continue
            extra = {}
            for ins in heavy:
                w = list(ins.sync_info.on_wait)
                ins.sync_info.on_wait = w[:self.MAX_WAITS]
                nops = []
                for i in range(self.MAX_WAITS, len(w), self.MAX_WAITS):
                    n = nc.engines[ins.engine].nop(hint="wait_split")
                    # pop the freshly appended nop from wherever it landed
                    for bb2 in nc.main_func.blocks:
                        if bb2.instructions and bb2.instructions[-1] is n.ins:
                            bb2.instructions.pop()
                            break
                    for wt in w[i:i + self.MAX_WAITS]:
                        handle = _bass_rust.SemaphoreHandle(wt.ant_name, wt.id)
                        _bass_rust.wait_op(n.ins, handle, wt.wait_value,
                                           "sem-ge", False)
                    nops.append(n.ins)
                extra[id(ins)] = nops
            new_list = []
            for ins in bb.instructions:
                new_list.extend(extra.get(id(ins), ()))
                new_list.append(ins)
            bb.instructions[:] = new_list

    def _drain_and_barrier(self, tick_clock, wait_clock):
        nc = self.nc
        self._split_body_waits()
        probe = nc.sync.nop(hint="tile_exit_wait_0")
        wait_clock.add_sem_waits(
            probe.ins, ScopedClock({None: tick_clock.global_clock})
        )
        waits = list(probe.ins.sync_info.on_wait or [])
        probe.ins.sync_info.on_wait = waits[:1]
        for w in waits[1:]:
            n = nc.sync.nop(hint="tile_exit_wait")
            handle = _bass_rust.SemaphoreHandle(w.ant_name, w.id)
            _bass_rust.wait_op(n.ins, handle, w.wait_value, "sem-ge", False)
        nc.sync.drain()
        nc.all_engine_barrier()
        assert self.sems is not None
        popped = nc._tile_sem_poison_stack.pop()
        assert popped is self._sem_poison
        nc.clear_and_free_semaphores(list(self.sems.allocated().values()))
        nc.all_engine_barrier()


def _ap(t, extra, *dims):
    """AP over tile t's buffer: partition dim from the tile, custom free dims.

    dims[0] may override the partition [step, count]."""
    base = t[:]
    return bass.AP(base.tensor, base.offset + extra, list(dims))


def _w0_perm():
    """Row permutation p s.t. w0_permuted[i] = w0[p[i]] matches the kernel's
    feature order: per oc: [S1(9) | S2(81)] then rows 90 + ij*90 + c with
    c<81 -> level4 (ij,kl=c), c>=81 -> level3 (ij, k=c-81)."""
    p = np.empty(OUT_CH * SIGC, np.int64)
    i = 0
    for oc in range(OUT_CH):
        base = oc * SIGC
        p[i:i + 90] = base + np.arange(90)          # S1 then S2, native order
        i += 90
        for ij in range(81):
            # level-4 block (ij, kl) for kl in 0..80 -> orig 819 + ij*81 + kl
            p[i:i + 81] = base + 819 + ij * 81 + np.arange(81)
            i += 81
            # level-3 (ij, k) for k in 0..8 -> orig 90 + ij*9 + k
            p[i:i + 9] = base + 90 + ij * 9 + np.arange(9)
            i += 9
    assert i == OUT_CH * SIGC
    return p


def _build():
    nc = bass.Bass(num_devices=NCORES, target_bir_lowering=True, trn_type="TRN2")

    # ---- per-core DRAM inputs ----
    xs = nc.dram_tensor("xs", [T, BL, IN_CH], F32, kind="ExternalInput")
    cwr = nc.dram_tensor("cwr", [128, 16], F32, kind="ExternalInput")
    cbr = nc.dram_tensor("cbr", [128, OUT_CH], F32, kind="ExternalInput")
    tlin = nc.dram_tensor("tlin", [128, 1], F32, kind="ExternalInput")
    ltri = nc.dram_tensor("ltri", [128, 128], F32R, kind="ExternalInput")
    onef = nc.dram_tensor("onef", [128, 1], F32, kind="ExternalInput")
    oner = nc.dram_tensor("oner", [128, 1], F32R, kind="ExternalInput")
    idn = nc.dram_tensor("idn", [128, 128], F32, kind="ExternalInput")
    w0s = nc.dram_tensor("w0s", [KSH, H0], BF16, kind="ExternalInput")
    w1s = nc.dram_tensor("w1s", [H0, H1], F32, kind="ExternalInput")
    w2s = nc.dram_tensor("w2s", [H1, NCLS], F32, kind="ExternalInput")
    b0c = nc.dram_tensor("b0c", [H0, 1], F32, kind="ExternalInput")
    b1c = nc.dram_tensor("b1c", [H1, 1], F32, kind="ExternalInput")
    b2r = nc.dram_tensor("b2r", [BL, NCLS], F32, kind="ExternalInput")
    out = nc.dram_tensor("out", [BL, NCLS], F32, kind="ExternalOutput")

    with _SplitDrainTileContext(nc) as tc:
        with tc.tile_pool(name="dram", bufs=1, space="DRAM") as dram:
            zl = dram.tile([NCORES * KSH, BL], BF16)     # local z^T, f'-order
            zex = dram.tile([NCORES * KSH, BL], BF16)    # post-A2A shard
            cin = dram.tile([B, H0], F32)
            cout = dram.tile([BL, H0], F32)
            prow = dram.tile([1, LANES * D], F32)        # p[T-1] bounce

            with tc.tile_pool(name="const", bufs=1) as cpool, \
                 tc.tile_pool(name="w0p", bufs=1) as w0pool, \
                 tc.tile_pool(name="prep", bufs=1) as ppool:
                lt_sb = cpool.tile([128, 128], F32R)
                nc.sync.dma_start(lt_sb[:], ltri[:])
                onef_sb = cpool.tile([128, 1], F32)
                nc.sync.dma_start(onef_sb[:], onef[:])
                oner_sb = cpool.tile([128, 1], F32R)
                nc.sync.dma_start(oner_sb[:], oner[:])
                tl_sb = cpool.tile([128, 1], F32)
                nc.sync.dma_start(tl_sb[:], tlin[:])
                cw_sb = cpool.tile([128, 16], F32)
                nc.sync.dma_start(cw_sb[:], cwr[:])
                cb_sb = cpool.tile([128, OUT_CH], F32)
                nc.sync.dma_start(cb_sb[:], cbr[:])
                idn_sb = cpool.tile([128, 128], F32)
                nc.sync.dma_start(idn_sb[:], idn[:])
                w1_sb = [cpool.tile([128, H1], F32, tag=f"w1_{j}", name=f"w1_{j}")
                         for j in range(4)]
                for j in range(4):
                    nc.sync.dma_start(w1_sb[j][:], w1s[j * 128:(j + 1) * 128, :])
                w2_sb = [cpool.tile([128, NCLS], F32, tag=f"w2_{j}", name=f"w2_{j}")
                         for j in range(2)]
                for j in range(2):
                    nc.sync.dma_start(w2_sb[j][:], w2s[j * 128:(j + 1) * 128, :])
                b0_sb = cpool.tile([128, 4], F32)
                nc.sync.dma_start(b0_sb[:], _ap(b0c, 0, [1, 128], [128, 4]))
                b1_sb = cpool.tile([128, 2], F32)
                nc.sync.dma_start(b1_sb[:], _ap(b1c, 0, [1, 128], [128, 2]))
                b2_sb = cpool.tile([BL, NCLS], F32)
                nc.sync.dma_start(b2_sb[:], b2r[:])
                xs_sb = cpool.tile([128, BL * IN_CH], F32)
                nc.sync.dma_start(xs_sb[:], _ap(xs, 0, [BL * IN_CH, 128],
                                                [1, BL * IN_CH]))
                # ---- prep: conv -> path p, then dx, P1, Y, u, u2, A ----
                W = LANES * D  # 288
                p = ppool.tile([128, W], F32, tag="p")
                dx = ppool.tile([128, W], F32, tag="dx")
                p1 = ppool.tile([128, W], F32, tag="p1")
                yt = ppool.tile([128, W], F32, tag="yt")
                pl = ppool.tile([128, W], F32, tag="pl")
                ut = ppool.tile([128, W], F32, tag="ut")
                u2 = ppool.tile([128, W], F32, tag="u2")
                at = ppool.tile([128, W], F32, tag="at")

                for oc in range(OUT_CH):
                    dst = _ap(p, oc * D + 1, [W, 128], [4 * D, BL], [1, CH])
                    for k in range(4):
                        src = _ap(xs_sb, k, [BL * IN_CH, 128], [IN_CH, BL], [4, CH])
                        wsc = cw_sb[:, oc * 4 + k:oc * 4 + k + 1]
                        if k == 0:
                            nc.vector.tensor_scalar(dst, src, wsc, None, AL.mult)
                        else:
                            nc.vector.scalar_tensor_tensor(
                                dst, src, wsc, dst, AL.mult, AL.add)
                    nc.vector.tensor_scalar(dst, dst,
                                            cb_sb[:, oc:oc + 1], None, AL.add)
                # time channel into col 0 of every lane
                nc.vector.tensor_copy(_ap(p, 0, [W, 128], [D, LANES]),
                                      _ap(tl_sb, 0, [1, 128], [0, LANES]))
                # P1 = p shifted down one step (DMA: compute engines cannot
                # address unaligned partition bases), then dx = p - P1.
                nc.gpsimd.memset(p1[0:1, :], 0.0)
                nc.gpsimd.dma_start(p1[1:128, :], p[0:127, :])
                nc.vector.tensor_tensor(dx[:], p[:], p1[:], AL.subtract)
                # Y[t] = p[T-1] - p[t]  (broadcast last row via DRAM bounce)
                nc.gpsimd.dma_start(prow[:], p[127:128, :])
                nc.gpsimd.dma_start(pl[:], _ap(prow, 0, [0, 128], [1, W]))
                nc.vector.tensor_tensor(yt[:], pl[:], p[:], AL.subtract)
                # ut = P1/2 + dx/6, u2 = P1/3 + dx/12, A = P1 + dx/2,
                # dxh = dx/2 (scalars pre-folded so the group outer products
                # are plain TensorTensor ops)
                nc.vector.tensor_scalar(ut[:], p1[:], 0.5, None, AL.mult)
                nc.vector.scalar_tensor_tensor(ut[:], dx[:], 1.0 / 6, ut[:],
                                               AL.mult, AL.add)
                nc.vector.tensor_scalar(u2[:], p1[:], 1.0 / 3, None, AL.mult)
                nc.vector.scalar_tensor_tensor(u2[:], dx[:], 1.0 / 12, u2[:],
                                               AL.mult, AL.add)
                nc.vector.scalar_tensor_tensor(at[:], dx[:], 0.5, p1[:],
                                               AL.mult, AL.add)
                dxh = ppool.tile([128, W], F32, tag="dxh")
                nc.vector.tensor_scalar(dxh[:], dx[:], 0.5, None, AL.mult)

                # ---- per-lane feature tiles, filled group by group ----
                zt4all = cpool.tile([81, OUT_CH * 720], BF16, tag="zt4all",
                                    name="zt4all")
                z1all = cpool.tile([9, LANES], BF16, tag="z1all", name="z1all")
                z2all = cpool.tile([81, LANES], BF16, tag="z2all", name="z2all")

                with tc.tile_pool(name="grp", bufs=2) as gpool, \
                     tc.tile_pool(name="ps2", bufs=2, space="PSUM") as ps2, \
                     tc.tile_pool(name="ptab", bufs=1, space="PSUM") as ptab, \
                     tc.tile_pool(name="ps12", bufs=1, space="PSUM") as ps12:
                    for g in range(NG):
                        off = g * GL * D  # col offset into the 288-wide tiles
                        GW = GL * 81      # 648

                        def o_ij(t, st=1):  # [lane, i(step), j(bcast)] view
                            return _ap(t, off, [W, 128], [D, GL], [st, D], [0, D])

                        def o_ji(t, st=1):  # [lane, i(bcast), j(step)] view
                            return _ap(t, off, [W, 128], [D, GL], [0, D], [st, D])

                        m2 = gpool.tile([128, GW], F32R, tag="m2")
                        m2v = _ap(m2, 0, [GW, 128], [81, GL], [D, D], [1, D])
                        nc.vector.tensor_tensor(m2v, o_ij(at), o_ji(dx), AL.mult)
                        # fp32 alias for the N=1 column-sum matmul (f32r
                        # matmuls reject N=1 in this walrus build)
                        m2f = gpool.tile([128, GW], F32, tag="m2f")
                        nc.vector.tensor_copy(m2f[:], m2[:])

                        s2 = ps2.tile([128, GW], F32, tag="s2")
                        nc.tensor.matmul(s2[:, 0:512], lt_sb[:], m2[:, 0:512],
                                         start=True, stop=True)
                        nc.tensor.matmul(s2[:, 512:GW], lt_sb[:], m2[:, 512:GW],
                                         start=True, stop=True)

                        bt = gpool.tile([128, GW], F32R, tag="bt")
                        btv = _ap(bt, 0, [GW, 128], [81, GL], [D, D], [1, D])
                        nc.vector.tensor_tensor(btv, o_ij(ut), o_ji(dx), AL.mult)
                        nc.vector.tensor_tensor(bt[:], bt[:], s2[:], AL.add)

                        t8 = gpool.tile([128, GW], F32R, tag="t8")
                        t8v = _ap(t8, 0, [GW, 128], [81, GL], [D, D], [1, D])
                        nc.vector.tensor_tensor(t8v, o_ij(u2), o_ji(dx), AL.mult)
                        nc.vector.tensor_tensor(t8[:], t8[:], s2[:], AL.add)

                        # 82-wide lane stride: f32r matmuls need even N,
                        # so rhs is [128, 82] with a zero 82nd column
                        q2 = gpool.tile([128, GL * 82], F32R, tag="q2")
                        q2v = _ap(q2, 0, [GL * 82, 128], [82, GL], [D, D], [1, D])
                        nc.vector.tensor_tensor(q2v, o_ij(dxh), o_ji(dx), AL.mult)
                        nc.vector.tensor_scalar(
                            _ap(q2, 81, [GL * 82, 128], [82, GL]),
                            _ap(dxh, 0, [LANES * D, 128], [0, GL]),
                            0.0, None, AL.mult)

                        rx = gpool.tile([128, GL * 90], F32R, tag="rx")
                        rxv = _ap(rx, 0, [GL * 90, 128], [90, GL], [D, D], [1, D])
                        nc.vector.tensor_tensor(rxv, o_ij(dx), o_ji(yt), AL.mult)
                        nc.vector.tensor_copy(
                            _ap(rx, 81, [GL * 90, 128], [90, GL], [1, D]),
                            _ap(dx, off, [W, 128], [D, GL], [1, D]))

                        tab = ptab.tile([128, 1024], F32, tag="tab")
                        s12 = ps12.tile([128, 2 * GL], F32, tag="s12")
                        for l in range(GL):
                            lane = g * GL + l
                            bloc, oc = lane // OUT_CH, lane % OUT_CH
                            nc.tensor.matmul(
                                _ap(tab, 128 * l, [1024, 81], [1, 90]),
                                bt[:, l * 81:(l + 1) * 81],
                                rx[:, l * 90:(l + 1) * 90],
                                start=True, stop=False)
                            nc.tensor.matmul(
                                _ap(tab, 128 * l, [1024, 81], [1, 82]),
                                t8[:, l * 81:(l + 1) * 81],
                                q2[:, l * 82:(l + 1) * 82],
                                start=False, stop=True)
                            nc.tensor.matmul(s12[0:9, 2 * l:2 * l + 1],
                                             dx[:, lane * D:(lane + 1) * D],
                                             onef_sb[:], start=True, stop=True)
                            nc.tensor.matmul(s12[0:81, 2 * l + 1:2 * l + 2],
                                             m2f[:, l * 81:(l + 1) * 81],
                                             onef_sb[:], start=True, stop=True)
                        # batched PSUM->SBUF evacuation for the whole group:
                        # lanes l = bo*4 + oc, local batch bloc = 2g + bo
                        nc.vector.tensor_copy(
                            _ap(zt4all, 2 * g, [2880, 81],
                                [1, 2], [720, OUT_CH], [8, 90]),
                            _ap(tab, 0, [1024, 81],
                                [512, 2], [128, OUT_CH], [1, 90]))
                        nc.scalar.activation(
                            _ap(z1all, 2 * g, [LANES, 9], [1, 2], [8, OUT_CH]),
                            _ap(s12, 0, [2 * GL, 9], [8, 2], [2, OUT_CH]),
                            AF.Copy)
                        nc.scalar.activation(
                            _ap(z2all, 2 * g, [LANES, 81], [1, 2], [8, OUT_CH]),
                            _ap(s12, 1, [2 * GL, 81], [8, 2], [2, OUT_CH]),
                            AF.Copy)

                # ---- pack z^T shard-ordered into DRAM and exchange ----
                for oc in range(OUT_CH):
                    base = oc * SIGC * BL
                    nc.scalar.dma_start(
                        _ap(zl, base, [8, 9], [1, 8]),
                        z1all[:, oc * BL:(oc + 1) * BL])
                    nc.scalar.dma_start(
                        _ap(zl, base + 9 * BL, [8, 81], [1, 8]),
                        z2all[:, oc * BL:(oc + 1) * BL])
                    nc.scalar.dma_start(
                        _ap(zl, base + 90 * BL, [720, 81], [8, 90], [1, 8]),
                        zt4all[:, oc * 720:(oc + 1) * 720])
                # w0 shard prefetch (emitted here so these no-dep DMAs sit
                # behind nothing critical on the sync queue, and stream
                # during the signature phase / AllToAll)
                w0_t = []
                for i in range(29):
                    k0 = i * 128
                    kn = min(128, KSH - k0)
                    wt = w0pool.tile([128, H0], BF16, tag=f"w0_{i}", name=f"w0_{i}")
                    nc.sync.dma_start(wt[0:kn, :], w0s[k0:k0 + kn, :])
                    w0_t.append((wt, k0, kn))
                nc.gpsimd.collective_compute(
                    "AllToAll", AL.bypass,
                    replica_groups=[list(range(NCORES))],
                    ins=[zl[:].opt()], outs=[zex[:].opt()])

                # ---- z0 = z @ w0 partial over this core's K shard ----
                with tc.tile_pool(name="zex", bufs=4) as zpool, \
                     tc.tile_pool(name="pz0", bufs=1, space="PSUM") as pz0p, \
                     tc.tile_pool(name="ptail", bufs=1, space="PSUM") as ptail:
                    z0p = pz0p.tile([B, H0], F32, tag="z0p")
                    for i, (wt, k0, kn) in enumerate(w0_t):
                        ztl = zpool.tile([128, B], BF16, tag="ztl")
                        eng = nc.scalar if i % 2 else nc.sync
                        eng.dma_start(
                            ztl[0:kn, :],
                            _ap(zex, k0 * BL, [BL, kn],
                                [KSH * BL, NCORES], [1, BL]))
                        nc.tensor.matmul(z0p[:], ztl[0:kn, :], wt[0:kn, :],
                                         start=(i == 0), stop=(i == 28))
                    z0sb = cpool.tile([B, H0], F32, tag="z0sb", name="z0sb")
                    nc.vector.tensor_copy(z0sb[:], z0p[:])
                    nc.sync.dma_start(cin[:], z0sb[:])
                    nc.gpsimd.collective_compute(
                        "ReduceScatter", AL.add,
                        replica_groups=[list(range(NCORES))],
                        ins=[cin[:].opt()], outs=[cout[:].opt()])

                    # ---- tail: sigmoid(w0 out) -> w1 -> sigmoid -> w2 ----
                    z1row = cpool.tile([BL, H0], F32, tag="z1row")
                    nc.sync.dma_start(z1row[:], cout[:])
                    pz1 = ptail.tile([128, 4 * BL], F32, tag="pz1")
                    z1t = cpool.tile([128, 4 * BL], F32, tag="z1t")
                    for j in range(4):
                        nc.tensor.transpose(pz1[:, j * BL:(j + 1) * BL],
                                            z1row[:, j * 128:(j + 1) * 128],
                                            idn_sb[0:BL, 0:BL])
                        nc.scalar.activation(z1t[:, j * BL:(j + 1) * BL],
                                             pz1[:, j * BL:(j + 1) * BL],
                                             AF.Sigmoid, bias=b0_sb[:, j:j + 1])
                    pz2 = ptail.tile([128, 2 * BL], F32, tag="pz2")
                    z2t = cpool.tile([128, 2 * BL], F32, tag="z2t")
                    for m in range(2):
                        for kj in range(4):
                            nc.tensor.matmul(
                                pz2[:, m * BL:(m + 1) * BL],
                                w1_sb[kj][:, m * 128:(m + 1) * 128],
                                z1t[:, kj * BL:(kj + 1) * BL],
                                start=(kj == 0), stop=(kj == 3))
                        nc.scalar.activation(z2t[:, m * BL:(m + 1) * BL],
                                             pz2[:, m * BL:(m + 1) * BL],
                                             AF.Sigmoid, bias=b1_sb[:, m:m + 1])
                    pz3 = ptail.tile([BL, NCLS], F32, tag="pz3")
                    for m in range(2):
                        nc.tensor.matmul(pz3[:], z2t[:, m * BL:(m + 1) * BL],
                                         w2_sb[m][:], start=(m == 0),
                                         stop=(m == 1))
                    z3 = cpool.tile([BL, NCLS], F32, tag="z3")
                    nc.vector.tensor_tensor(z3[:], pz3[:], b2_sb[:], AL.add)
                    mx = cpool.tile([BL, 1], F32, tag="mx")
                    nc.vector.tensor_reduce(mx[:], z3[:], mybir.AxisListType.X,
                                            AL.max)
                    tm = cpool.tile([BL, NCLS], F32, tag="tm")
                    nc.vector.tensor_scalar(tm[:], z3[:], mx[:, 0:1], None,
                                            AL.subtract)
                    ex = cpool.tile([BL, NCLS], F32, tag="ex")
                    se = cpool.tile([BL, 1], F32, tag="se")
                    nc.scalar.activation(ex[:], tm[:], AF.Exp, accum_out=se[:])
                    ls = cpool.tile([BL, 1], F32, tag="ls")
                    nc.scalar.activation(ls[:], se[:], AF.Ln)
                    osb = cpool.tile([BL, NCLS], F32, tag="osb")
                    nc.vector.tensor_scalar(osb[:], tm[:], ls[:, 0:1], None,
                                            AL.subtract)
                    nc.sync.dma_start(out[:], osb[:])
    return nc


_CACHE = {}


def kernel(x, conv_w, conv_b, w0, b0, w1, b1, w2, b2):
    x = np.ascontiguousarray(np.asarray(x, np.float32))
    conv_w = np.asarray(conv_w, np.float32)
    conv_b = np.asarray(conv_b, np.float32)
    w0 = np.asarray(w0, np.float32)
    w1 = np.ascontiguousarray(np.asarray(w1, np.float32))
    w2 = np.ascontiguousarray(np.asarray(w2, np.float32))
    b0 = np.asarray(b0, np.float32)
    b1 = np.asarray(b1, np.float32)
    b2 = np.asarray(b2, np.float32)

    if "nc" not in _CACHE:
        _CACHE["nc"] = _build()
        _CACHE["perm"] = _w0_perm()
    nc = _CACHE["nc"]
    bf16 = mybir.dt.np(BF16)
    w0p = np.ascontiguousarray(w0[_CACHE["perm"], :]).astype(bf16)

    shared = {
        "cwr": np.ascontiguousarray(
            np.broadcast_to(conv_w.reshape(1, 16), (128, 16))),
        "cbr": np.ascontiguousarray(
            np.broadcast_to(conv_b.reshape(1, OUT_CH), (128, OUT_CH))),
        "tlin": np.linspace(0.0, 1.0, T, dtype=np.float32).reshape(128, 1),
        "ltri": np.ascontiguousarray(
            np.triu(np.ones((128, 128), np.float32), 1)),
        "onef": np.ones((128, 1), np.float32),
        "oner": np.ones((128, 1), np.float32),
        "idn": np.eye(128, dtype=np.float32),
        "w1s": w1, "w2s": w2,
        "b0c": b0.reshape(H0, 1), "b1c": b1.reshape(H1, 1),
        "b2r": np.ascontiguousarray(np.broadcast_to(b2.reshape(1, NCLS),
                                                    (BL, NCLS))),
    }
    in_maps = []
    for c in range(NCORES):
        m = dict(shared)
        m["xs"] = np.ascontiguousarray(
            x[c * BL:(c + 1) * BL, 0].transpose(1, 0, 2))
        m["w0s"] = np.ascontiguousarray(w0p[c * KSH:(c + 1) * KSH, :])
        in_maps.append(m)

    _CACHE["in_maps"] = in_maps
    res = run_bass_kernel_spmd(nc, in_maps, core_ids=list(range(NCORES)))
    return np.concatenate([res.results[c]["out"] for c in range(NCORES)],
                          axis=0)


# revision 28
# speedup vs baseline: 1.2989x; 1.0240x over previous
"""CNN + truncated path-signature (depth 4) + FF head on 8 TRN2 NeuronCores.

Strategy
--------
- Batch data-parallel signature computation: core c handles batches
  [8c, 8c+8) = 32 (batch, out_ch) lanes, T=128 time steps on partitions.
- Signature reformulated to avoid sequential scans:
    dx, P1 (= shifted path), Y (= suffix sums) are free;
    the only prefix scan (level 2, S2pre) is one triangular matmul L @ M2;
    S3 = B^T X;  S4 = B^T R + tmp81'^T (dx(x)dx)/2   (suffix-vector trick),
  so levels 3 and 4 are plain T-contractions on the TensorEngine.
- The 60 MB w0 is row-sharded 8 ways (this is the memory roofline win):
  AllToAll re-shards the signature activations feature-wise, each core
  multiplies its [3690, 512] w0 shard, ReduceScatter returns each core its
  own 8 batches, and the small w1/w2 head finishes per-core.
- w0 rows are permuted host-side to match the kernel's natural feature
  order, so no on-device transposes are needed.
"""
import os
import sys
sys.path.insert(0, "/opt/trn_rl_repo")
if os.environ.get("JAX_PLATFORMS") == "cpu":
    # The SPMD launch needs the axon/neuron PJRT backend.
    os.environ["JAX_PLATFORMS"] = ""

import numpy as np
import bass_rust as _bass_rust
import concourse.bass as bass
import concourse.tile as tile
import concourse.mybir as mybir
from concourse.vector_clock import ScopedClock
from concourse.bass_utils import run_bass_kernel_spmd

F32 = mybir.dt.float32
F32R = mybir.dt.float32r
BF16 = mybir.dt.bfloat16
AL = mybir.AluOpType
AF = mybir.ActivationFunctionType

NCORES = 8
B, T, IN_CH = 64, 128, 32
OUT_CH, CH, D = 4, 8, 9            # conv out-channels, conv width, path dim
BL = B // NCORES                   # local batches = 8
LANES = BL * OUT_CH                # 32 lanes/core
NG = 4                             # lane groups of 8
GL = 8                             # lanes per group
SIGC = 7380                        # per-lane signature channels
KSH = OUT_CH * SIGC // NCORES      # w0 K-shard rows per core = 3690
H0, H1, NCLS = 512, 256, 10


class _SplitDrainTileContext(tile.TileContext):
    """Tile exit drain carries one sem wait per CTRL instruction.

    This container's walrus build rejects >2 sync waits on a CTRL
    instruction; Tile's exit drain waits on the whole global clock.
    Redistribute the waits over nops on the same engine (program order on
    one engine preserves semantics)."""

    MAX_WAITS = 1

    def _split_body_waits(self):
        """Move excess sem waits from any instruction onto preceding nops on
        the same engine (same-engine program order preserves semantics)."""
        nc = self.nc
        for bb in nc.main_func.blocks:
            heavy = [ins for ins in bb.instructions
                     if ins.sync_info and ins.sync_info.on_wait
                     and len(ins.sync_info.on_wait) > self.MAX_WAITS]
            if not heavy:
                continue
            extra = {}
            for ins in heavy:
                w = list(ins.sync_info.on_wait)
                ins.sync_info.on_wait = w[:self.MAX_WAITS]
                nops = []
                for i in range(self.MAX_WAITS, len(w), self.MAX_WAITS):
                    n = nc.engines[ins.engine].nop(hint="wait_split")
                    # pop the freshly appended nop from wherever it landed
                    for bb2 in nc.main_func.blocks:
                        if bb2.instructions and bb2.instructions[-1] is n.ins:
                            bb2.instructions.pop()
                            break
                    for wt in w[i:i + self.MAX_WAITS]:
                        handle = _bass_rust.SemaphoreHandle(wt.ant_name, wt.id)
                        _bass_rust.wait_op(n.ins, handle, wt.wait_value,
                                           "sem-ge", False)
                    nops.append(n.ins)
                extra[id(ins)] = nops
            new_list = []
            for ins in bb.instructions:
                new_list.extend(extra.get(id(ins), ()))
                new_list.append(ins)
            bb.instructions[:] = new_list

    def _drain_and_barrier(self, tick_clock, wait_clock):
        nc = self.nc
        self._split_body_waits()
        probe = nc.sync.nop(hint="tile_exit_wait_0")
        wait_clock.add_sem_waits(
            probe.ins, ScopedClock({None: tick_clock.global_clock})
        )
        waits = list(probe.ins.sync_info.on_wait or [])
        probe.ins.sync_info.on_wait = waits[:1]
        for w in waits[1:]:
            n = nc.sync.nop(hint="tile_exit_wait")
            handle = _bass_rust.SemaphoreHandle(w.ant_name, w.id)
            _bass_rust.wait_op(n.ins, handle, w.wait_value, "sem-ge", False)
        nc.sync.drain()
        nc.all_engine_barrier()
        assert self.sems is not None
        popped = nc._tile_sem_poison_stack.pop()
        assert popped is self._sem_poison
        nc.clear_and_free_semaphores(list(self.sems.allocated().values()))
        nc.all_engine_barrier()


def _ap(t, extra, *dims):
    """AP over tile t's buffer: partition dim from the tile, custom free dims.

    dims[0] may override the partition [step, count]."""
    base = t[:]
    return bass.AP(base.tensor, base.offset + extra, list(dims))


def _w0_perm():
    """Row permutation p s.t. w0_permuted[i] = w0[p[i]] matches the kernel's
    feature order: per oc: [S1(9) | S2(81)] then rows 90 + ij*90 + c with
    c<81 -> level4 (ij,kl=c), c>=81 -> level3 (ij, k=c-81)."""
    p = np.empty(OUT_CH * SIGC, np.int64)
    i = 0
    for oc in range(OUT_CH):
        base = oc * SIGC
        p[i:i + 90] = base + np.arange(90)          # S1 then S2, native order
        i += 90
        for ij in range(81):
            # level-4 block (ij, kl) for kl in 0..80 -> orig 819 + ij*81 + kl
            p[i:i + 81] = base + 819 + ij * 81 + np.arange(81)
            i += 81
            # level-3 (ij, k) for k in 0..8 -> orig 90 + ij*9 + k
            p[i:i + 9] = base + 90 + ij * 9 + np.arange(9)
            i += 9
    assert i == OUT_CH * SIGC
    return p


def _build():
    nc = bass.Bass(num_devices=NCORES, target_bir_lowering=True, trn_type="TRN2")

    # ---- per-core DRAM inputs ----
    xs = nc.dram_tensor("xs", [T, BL, IN_CH], F32, kind="ExternalInput")
    cwr = nc.dram_tensor("cwr", [128, 16], F32, kind="ExternalInput")
    cbr = nc.dram_tensor("cbr", [128, OUT_CH], F32, kind="ExternalInput")
    tlin = nc.dram_tensor("tlin", [128, 1], F32, kind="ExternalInput")
    ltri = nc.dram_tensor("ltri", [128, 128], F32R, kind="ExternalInput")
    onef = nc.dram_tensor("onef", [128, 1], F32, kind="ExternalInput")
    oner = nc.dram_tensor("oner", [128, 2], F32R, kind="ExternalInput")
    idn = nc.dram_tensor("idn", [128, 128], F32, kind="ExternalInput")
    w0s = nc.dram_tensor("w0s", [KSH, H0], BF16, kind="ExternalInput")
    w1s = nc.dram_tensor("w1s", [H0, H1], F32, kind="ExternalInput")
    w2s = nc.dram_tensor("w2s", [H1, NCLS], F32, kind="ExternalInput")
    b0c = nc.dram_tensor("b0c", [H0, 1], F32, kind="ExternalInput")
    b1c = nc.dram_tensor("b1c", [H1, 1], F32, kind="ExternalInput")
    b2r = nc.dram_tensor("b2r", [BL, NCLS], F32, kind="ExternalInput")
    out = nc.dram_tensor("out", [BL, NCLS], F32, kind="ExternalOutput")

    with _SplitDrainTileContext(nc) as tc:
        with tc.tile_pool(name="dram", bufs=1, space="DRAM") as dram:
            zl = dram.tile([NCORES * KSH, BL], BF16)     # local z^T, f'-order
            zex = dram.tile([NCORES * KSH, BL], BF16)    # post-A2A shard
            cin = dram.tile([B, H0], F32)
            cout = dram.tile([BL, H0], F32)
            prow = dram.tile([1, LANES * D], F32)        # p[T-1] bounce

            with tc.tile_pool(name="const", bufs=1) as cpool, \
                 tc.tile_pool(name="w0p", bufs=1) as w0pool, \
                 tc.tile_pool(name="prep", bufs=1) as ppool:
                lt_sb = cpool.tile([128, 128], F32R)
                nc.scalar.dma_start(lt_sb[:], ltri[:])
                onef_sb = cpool.tile([128, 1], F32)
                nc.scalar.dma_start(onef_sb[:], onef[:])
                oner_sb = cpool.tile([128, 2], F32R)
                nc.scalar.dma_start(oner_sb[:], oner[:])
                tl_sb = cpool.tile([128, 1], F32)
                nc.scalar.dma_start(tl_sb[:], tlin[:])
                cw_sb = cpool.tile([128, 16], F32)
                nc.scalar.dma_start(cw_sb[:], cwr[:])
                cb_sb = cpool.tile([128, OUT_CH], F32)
                nc.scalar.dma_start(cb_sb[:], cbr[:])
                idn_sb = cpool.tile([128, 128], F32)
                nc.scalar.dma_start(idn_sb[:], idn[:])
                w1_sb = [cpool.tile([128, H1], F32, tag=f"w1_{j}", name=f"w1_{j}")
                         for j in range(4)]
                for j in range(4):
                    nc.scalar.dma_start(w1_sb[j][:], w1s[j * 128:(j + 1) * 128, :])
                w2_sb = [cpool.tile([128, NCLS], F32, tag=f"w2_{j}", name=f"w2_{j}")
                         for j in range(2)]
                for j in range(2):
                    nc.scalar.dma_start(w2_sb[j][:], w2s[j * 128:(j + 1) * 128, :])
                b0_sb = cpool.tile([128, 4], F32)
                nc.scalar.dma_start(b0_sb[:], _ap(b0c, 0, [1, 128], [128, 4]))
                b1_sb = cpool.tile([128, 2], F32)
                nc.scalar.dma_start(b1_sb[:], _ap(b1c, 0, [1, 128], [128, 2]))
                b2_sb = cpool.tile([BL, NCLS], F32)
                nc.scalar.dma_start(b2_sb[:], b2r[:])
                xs_sb = cpool.tile([128, BL * IN_CH], F32)
                nc.scalar.dma_start(xs_sb[:], _ap(xs, 0, [BL * IN_CH, 128],
                                                [1, BL * IN_CH]))
                # ---- prep: conv -> path p, then dx, P1, Y, u, u2, A ----
                W = LANES * D  # 288
                p = ppool.tile([128, W], F32, tag="p")
                dx = ppool.tile([128, W], F32, tag="dx")
                p1 = ppool.tile([128, W], F32, tag="p1")
                yt = ppool.tile([128, W], F32, tag="yt")
                pl = ppool.tile([128, W], F32, tag="pl")
                ut = ppool.tile([128, W], F32, tag="ut")
                u2 = ppool.tile([128, W], F32, tag="u2")
                at = ppool.tile([128, W], F32, tag="at")

                for oc in range(OUT_CH):
                    dst = _ap(p, oc * D + 1, [W, 128], [4 * D, BL], [1, CH])
                    for k in range(4):
                        src = _ap(xs_sb, k, [BL * IN_CH, 128], [IN_CH, BL], [4, CH])
                        wsc = cw_sb[:, oc * 4 + k:oc * 4 + k + 1]
                        if k == 0:
                            nc.vector.tensor_scalar(dst, src, wsc, None, AL.mult)
                        else:
                            nc.vector.scalar_tensor_tensor(
                                dst, src, wsc, dst, AL.mult, AL.add)
                    nc.vector.tensor_scalar(dst, dst,
                                            cb_sb[:, oc:oc + 1], None, AL.add)
                # time channel into col 0 of every lane
                nc.vector.tensor_copy(_ap(p, 0, [W, 128], [D, LANES]),
                                      _ap(tl_sb, 0, [1, 128], [0, LANES]))
                # P1 = p shifted down one step (DMA: compute engines cannot
                # address unaligned partition bases), then dx = p - P1.
                nc.gpsimd.memset(p1[0:1, :], 0.0)
                nc.gpsimd.dma_start(p1[1:128, :], p[0:127, :])
                nc.vector.tensor_tensor(dx[:], p[:], p1[:], AL.subtract)
                # Y[t] = p[T-1] - p[t]  (broadcast last row via DRAM bounce)
                nc.gpsimd.dma_start(prow[:], p[127:128, :])
                nc.gpsimd.dma_start(pl[:], _ap(prow, 0, [0, 128], [1, W]))
                nc.vector.tensor_tensor(yt[:], pl[:], p[:], AL.subtract)
                # ut = P1/2 + dx/6, u2 = P1/3 + dx/12, A = P1 + dx/2,
                # dxh = dx/2 (scalars pre-folded so the group outer products
                # are plain TensorTensor ops)
                nc.vector.tensor_scalar(ut[:], p1[:], 0.5, None, AL.mult)
                nc.vector.scalar_tensor_tensor(ut[:], dx[:], 1.0 / 6, ut[:],
                                               AL.mult, AL.add)
                nc.vector.tensor_scalar(u2[:], p1[:], 1.0 / 3, None, AL.mult)
                nc.vector.scalar_tensor_tensor(u2[:], dx[:], 1.0 / 12, u2[:],
                                               AL.mult, AL.add)
                nc.vector.scalar_tensor_tensor(at[:], dx[:], 0.5, p1[:],
                                               AL.mult, AL.add)
                dxh = ppool.tile([128, W], F32, tag="dxh")
                nc.vector.tensor_scalar(dxh[:], dx[:], 0.5, None, AL.mult)
                # preload ACT function tables off the critical tail
                dum = ppool.tile([1, 4], F32, tag="dum")
                nc.scalar.activation(dum[0:1, 0:1], tl_sb[0:1, 0:1], AF.Sigmoid)
                nc.scalar.activation(dum[0:1, 1:2], tl_sb[0:1, 0:1], AF.Exp)
                nc.scalar.activation(dum[0:1, 2:3], onef_sb[0:1, 0:1], AF.Ln)

                # ---- per-lane feature tiles, filled group by group ----
                zt4all = cpool.tile([81, OUT_CH * 720], BF16, tag="zt4all",
                                    name="zt4all")
                z1all = cpool.tile([9, LANES], BF16, tag="z1all", name="z1all")
                z2all = cpool.tile([81, LANES], BF16, tag="z2all", name="z2all")

                with tc.tile_pool(name="grp", bufs=2) as gpool, \
                     tc.tile_pool(name="ps2", bufs=2, space="PSUM") as ps2, \
                     tc.tile_pool(name="ptab", bufs=1, space="PSUM") as ptab, \
                     tc.tile_pool(name="ps12", bufs=1, space="PSUM") as ps12:
                    for g in range(NG):
                        off = g * GL * D  # col offset into the 288-wide tiles
                        GW = GL * 81      # 648

                        def o_ij(t, st=1):  # [lane, i(step), j(bcast)] view
                            return _ap(t, off, [W, 128], [D, GL], [st, D], [0, D])

                        def o_ji(t, st=1):  # [lane, i(bcast), j(step)] view
                            return _ap(t, off, [W, 128], [D, GL], [0, D], [st, D])

                        m2 = gpool.tile([128, GW], F32R, tag="m2")
                        m2v = _ap(m2, 0, [GW, 128], [81, GL], [D, D], [1, D])
                        nc.vector.tensor_tensor(m2v, o_ij(at), o_ji(dx), AL.mult)


                        s2 = ps2.tile([128, GW], F32, tag="s2")
                        nc.tensor.matmul(s2[:, 0:512], lt_sb[:], m2[:, 0:512],
                                         start=True, stop=True)
                        nc.tensor.matmul(s2[:, 512:GW], lt_sb[:], m2[:, 512:GW],
                                         start=True, stop=True)

                        bt = gpool.tile([128, GW], BF16, tag="bt")
                        btv = _ap(bt, 0, [GW, 128], [81, GL], [D, D], [1, D])
                        nc.vector.tensor_tensor(btv, o_ij(ut), o_ji(dx), AL.mult)
                        nc.vector.tensor_tensor(bt[:], bt[:], s2[:], AL.add)

                        t8 = gpool.tile([128, GW], BF16, tag="t8")
                        t8v = _ap(t8, 0, [GW, 128], [81, GL], [D, D], [1, D])
                        nc.vector.tensor_tensor(t8v, o_ij(u2), o_ji(dx), AL.mult)
                        nc.vector.tensor_tensor(t8[:], t8[:], s2[:], AL.add)

                        q2 = gpool.tile([128, GW], BF16, tag="q2")
                        q2v = _ap(q2, 0, [GW, 128], [81, GL], [D, D], [1, D])
                        nc.vector.tensor_tensor(q2v, o_ij(dxh), o_ji(dx), AL.mult)

                        rx = gpool.tile([128, GL * 90], BF16, tag="rx")
                        rxv = _ap(rx, 0, [GL * 90, 128], [90, GL], [D, D], [1, D])
                        nc.vector.tensor_tensor(rxv, o_ij(dx), o_ji(yt), AL.mult)
                        nc.vector.tensor_copy(
                            _ap(rx, 81, [GL * 90, 128], [90, GL], [1, D]),
                            _ap(dx, off, [W, 128], [D, GL], [1, D]))

                        tab = ptab.tile([128, 1024], F32, tag="tab")
                        s12 = ps12.tile([128, 4 * GL], F32, tag="s12")
                        for l in range(GL):
                            lane = g * GL + l
                            bloc, oc = lane // OUT_CH, lane % OUT_CH
                            nc.tensor.matmul(
                                _ap(tab, 128 * l, [1024, 81], [1, 90]),
                                bt[:, l * 81:(l + 1) * 81],
                                rx[:, l * 90:(l + 1) * 90],
                                start=True, stop=False)
                            nc.tensor.matmul(
                                _ap(tab, 128 * l, [1024, 81], [1, 81]),
                                t8[:, l * 81:(l + 1) * 81],
                                q2[:, l * 81:(l + 1) * 81],
                                start=False, stop=True)
                            nc.tensor.matmul(s12[0:9, 4 * l:4 * l + 1],
                                             dx[:, lane * D:(lane + 1) * D],
                                             onef_sb[:], start=True, stop=True)
                            nc.tensor.matmul(s12[0:81, 4 * l + 2:4 * l + 4],
                                             m2[:, l * 81:(l + 1) * 81],
                                             oner_sb[:], start=True, stop=True)
                        # batched PSUM->SBUF evacuation for the whole group:
                        # lanes l = bo*4 + oc, local batch bloc = 2g + bo
                        for bo in range(2):
                            nc.scalar.activation(
                                _ap(zt4all, 2 * g + bo, [2880, 81],
                                    [720, OUT_CH], [8, 90]),
                                _ap(tab, 512 * bo, [1024, 81],
                                    [128, OUT_CH], [1, 90]),
                                AF.Copy)
                        nc.scalar.activation(
                            _ap(z1all, 2 * g, [LANES, 9], [1, 2], [8, OUT_CH]),
                            _ap(s12, 0, [4 * GL, 9], [16, 2], [4, OUT_CH]),
                            AF.Copy)
                        nc.scalar.activation(
                            _ap(z2all, 2 * g, [LANES, 81], [1, 2], [8, OUT_CH]),
                            _ap(s12, 2, [4 * GL, 81], [16, 2], [4, OUT_CH]),
                            AF.Copy)

                # ---- pack z^T shard-ordered into DRAM and exchange ----
                for oc in range(OUT_CH):
                    base = oc * SIGC * BL
                    nc.scalar.dma_start(
                        _ap(zl, base, [8, 9], [1, 8]),
                        z1all[:, oc * BL:(oc + 1) * BL])
                    nc.scalar.dma_start(
                        _ap(zl, base + 9 * BL, [8, 81], [1, 8]),
                        z2all[:, oc * BL:(oc + 1) * BL])
                    nc.scalar.dma_start(
                        _ap(zl, base + 90 * BL, [720, 81], [8, 90], [1, 8]),
                        zt4all[:, oc * 720:(oc + 1) * 720])
                # w0 shard prefetch (emitted here so these no-dep DMAs sit
                # behind nothing critical on the sync queue, and stream
                # during the signature phase / AllToAll)
                w0_t = []
                for i in range(29):
                    k0 = i * 128
                    kn = min(128, KSH - k0)
                    wt = w0pool.tile([128, H0], BF16, tag=f"w0_{i}", name=f"w0_{i}")
                    nc.sync.dma_start(wt[0:kn, :], w0s[k0:k0 + kn, :])
                    w0_t.append((wt, k0, kn))
                nc.gpsimd.collective_compute(
                    "AllToAll", AL.bypass,
                    replica_groups=[list(range(NCORES))],
                    ins=[zl[:].opt()], outs=[zex[:].opt()])

                # ---- z0 = z @ w0 partial over this core's K shard ----
                with tc.tile_pool(name="zex", bufs=8) as zpool, \
                     tc.tile_pool(name="pz0", bufs=1, space="PSUM") as pz0p, \
                     tc.tile_pool(name="ptail", bufs=1, space="PSUM") as ptail:
                    z0p = pz0p.tile([B, H0], F32, tag="z0p")
                    for i, (wt, k0, kn) in enumerate(w0_t):
                        ztl = zpool.tile([128, B], BF16, tag="ztl")
                        eng = nc.scalar if i % 2 else nc.sync
                        eng.dma_start(
                            ztl[0:kn, :],
                            _ap(zex, k0 * BL, [BL, kn],
                                [KSH * BL, NCORES], [1, BL]))
                        nc.tensor.matmul(z0p[:], ztl[0:kn, :], wt[0:kn, :],
                                         start=(i == 0), stop=(i == 28))
                    z0sb = cpool.tile([B, H0], F32, tag="z0sb", name="z0sb")
                    nc.vector.tensor_copy(z0sb[:], z0p[:])
                    nc.sync.dma_start(cin[:], z0sb[:])
                    nc.gpsimd.collective_compute(
                        "ReduceScatter", AL.add,
                        replica_groups=[list(range(NCORES))],
                        ins=[cin[:].opt()], outs=[cout[:].opt()])

                    # ---- tail: sigmoid(w0 out) -> w1 -> sigmoid -> w2 ----
                    z1row = cpool.tile([BL, H0], F32, tag="z1row")
                    nc.sync.dma_start(z1row[:], cout[:])
                    pz1 = ptail.tile([128, 4 * BL], F32, tag="pz1")
                    z1t = cpool.tile([128, 4 * BL], F32, tag="z1t")
                    for j in range(4):
                        nc.tensor.transpose(pz1[:, j * BL:(j + 1) * BL],
                                            z1row[:, j * 128:(j + 1) * 128],
                                            idn_sb[0:BL, 0:BL])
                        nc.scalar.activation(z1t[:, j * BL:(j + 1) * BL],
                                             pz1[:, j * BL:(j + 1) * BL],
                                             AF.Sigmoid, bias=b0_sb[:, j:j + 1])
                    pz2 = ptail.tile([128, 2 * BL], F32, tag="pz2")
                    z2t = cpool.tile([128, 2 * BL], F32, tag="z2t")
                    for m in range(2):
                        for kj in range(4):
                            nc.tensor.matmul(
                                pz2[:, m * BL:(m + 1) * BL],
                                w1_sb[kj][:, m * 128:(m + 1) * 128],
                                z1t[:, kj * BL:(kj + 1) * BL],
                                start=(kj == 0), stop=(kj == 3))
                        nc.scalar.activation(z2t[:, m * BL:(m + 1) * BL],
                                             pz2[:, m * BL:(m + 1) * BL],
                                             AF.Sigmoid, bias=b1_sb[:, m:m + 1])
                    pz3 = ptail.tile([BL, NCLS], F32, tag="pz3")
                    for m in range(2):
                        nc.tensor.matmul(pz3[:], z2t[:, m * BL:(m + 1) * BL],
                                         w2_sb[m][:], start=(m == 0),
                                         stop=(m == 1))
                    z3 = cpool.tile([BL, NCLS], F32, tag="z3")
                    nc.vector.tensor_tensor(z3[:], pz3[:], b2_sb[:], AL.add)
                    mx = cpool.tile([BL, 1], F32, tag="mx")
                    nc.vector.tensor_reduce(mx[:], z3[:], mybir.AxisListType.X,
                                            AL.max)
                    tm = cpool.tile([BL, NCLS], F32, tag="tm")
                    nc.vector.tensor_scalar(tm[:], z3[:], mx[:, 0:1], None,
                                            AL.subtract)
                    ex = cpool.tile([BL, NCLS], F32, tag="ex")
                    se = cpool.tile([BL, 1], F32, tag="se")
                    nc.scalar.activation(ex[:], tm[:], AF.Exp, accum_out=se[:])
                    ls = cpool.tile([BL, 1], F32, tag="ls")
                    nc.scalar.activation(ls[:], se[:], AF.Ln)
                    osb = cpool.tile([BL, NCLS], F32, tag="osb")
                    nc.vector.tensor_scalar(osb[:], tm[:], ls[:, 0:1], None,
                                            AL.subtract)
                    nc.sync.dma_start(out[:], osb[:])
    return nc


_CACHE = {}


def kernel(x, conv_w, conv_b, w0, b0, w1, b1, w2, b2):
    x = np.ascontiguousarray(np.asarray(x, np.float32))
    conv_w = np.asarray(conv_w, np.float32)
    conv_b = np.asarray(conv_b, np.float32)
    w0 = np.asarray(w0, np.float32)
    w1 = np.ascontiguousarray(np.asarray(w1, np.float32))
    w2 = np.ascontiguousarray(np.asarray(w2, np.float32))
    b0 = np.asarray(b0, np.float32)
    b1 = np.asarray(b1, np.float32)
    b2 = np.asarray(b2, np.float32)

    if "nc" not in _CACHE:
        _CACHE["nc"] = _build()
        _CACHE["perm"] = _w0_perm()
    nc = _CACHE["nc"]
    bf16 = mybir.dt.np(BF16)
    w0p = np.ascontiguousarray(w0[_CACHE["perm"], :]).astype(bf16)

    shared = {
        "cwr": np.ascontiguousarray(
            np.broadcast_to(conv_w.reshape(1, 16), (128, 16))),
        "cbr": np.ascontiguousarray(
            np.broadcast_to(conv_b.reshape(1, OUT_CH), (128, OUT_CH))),
        "tlin": np.linspace(0.0, 1.0, T, dtype=np.float32).reshape(128, 1),
        "ltri": np.ascontiguousarray(
            np.triu(np.ones((128, 128), np.float32), 1)),
        "onef": np.ones((128, 1), np.float32),
        "oner": np.ones((128, 2), np.float32),
        "idn": np.eye(128, dtype=np.float32),
        "w1s": w1, "w2s": w2,
        "b0c": b0.reshape(H0, 1), "b1c": b1.reshape(H1, 1),
        "b2r": np.ascontiguousarray(np.broadcast_to(b2.reshape(1, NCLS),
                                                    (BL, NCLS))),
    }
    in_maps = []
    for c in range(NCORES):
        m = dict(shared)
        m["xs"] = np.ascontiguousarray(
            x[c * BL:(c + 1) * BL, 0].transpose(1, 0, 2))
        m["w0s"] = np.ascontiguousarray(w0p[c * KSH:(c + 1) * KSH, :])
        in_maps.append(m)

    _CACHE["in_maps"] = in_maps
    res = run_bass_kernel_spmd(nc, in_maps, core_ids=list(range(NCORES)))
    return np.concatenate([res.results[c]["out"] for c in range(NCORES)],
                          axis=0)


# revision 29
# speedup vs baseline: 1.3422x; 1.0333x over previous
"""CNN + truncated path-signature (depth 4) + FF head on 8 TRN2 NeuronCores.

Strategy
--------
- Batch data-parallel signature computation: core c handles batches
  [8c, 8c+8) = 32 (batch, out_ch) lanes, T=128 time steps on partitions.
- Signature reformulated to avoid sequential scans:
    dx, P1 (= shifted path), Y (= suffix sums) are free;
    the only prefix scan (level 2, S2pre) is one triangular matmul L @ M2;
    S3 = B^T X;  S4 = B^T R + tmp81'^T (dx(x)dx)/2   (suffix-vector trick),
  so levels 3 and 4 are plain T-contractions on the TensorEngine.
- The 60 MB w0 is row-sharded 8 ways (this is the memory roofline win):
  AllToAll re-shards the signature activations feature-wise, each core
  multiplies its [3690, 512] w0 shard, ReduceScatter returns each core its
  own 8 batches, and the small w1/w2 head finishes per-core.
- w0 rows are permuted host-side to match the kernel's natural feature
  order, so no on-device transposes are needed.
"""
import os
import sys
sys.path.insert(0, "/opt/trn_rl_repo")
if os.environ.get("JAX_PLATFORMS") == "cpu":
    # The SPMD launch needs the axon/neuron PJRT backend.
    os.environ["JAX_PLATFORMS"] = ""

import numpy as np
import bass_rust as _bass_rust
import concourse.bass as bass
import concourse.tile as tile
import concourse.mybir as mybir
from concourse.vector_clock import ScopedClock
from concourse.bass_utils import run_bass_kernel_spmd

F32 = mybir.dt.float32
F32R = mybir.dt.float32r
BF16 = mybir.dt.bfloat16
AL = mybir.AluOpType
AF = mybir.ActivationFunctionType

NCORES = 8
B, T, IN_CH = 64, 128, 32
OUT_CH, CH, D = 4, 8, 9            # conv out-channels, conv width, path dim
BL = B // NCORES                   # local batches = 8
LANES = BL * OUT_CH                # 32 lanes/core
NG = 4                             # lane groups of 8
GL = 8                             # lanes per group
SIGC = 7380                        # per-lane signature channels
KSH = OUT_CH * SIGC // NCORES      # w0 K-shard rows per core = 3690
H0, H1, NCLS = 512, 256, 10


class _SplitDrainTileContext(tile.TileContext):
    """Tile exit drain carries one sem wait per CTRL instruction.

    This container's walrus build rejects >2 sync waits on a CTRL
    instruction; Tile's exit drain waits on the whole global clock.
    Redistribute the waits over nops on the same engine (program order on
    one engine preserves semantics)."""

    MAX_WAITS = 1

    def _split_body_waits(self):
        """Move excess sem waits from any instruction onto preceding nops on
        the same engine (same-engine program order preserves semantics)."""
        nc = self.nc
        for bb in nc.main_func.blocks:
            heavy = [ins for ins in bb.instructions
                     if ins.sync_info and ins.sync_info.on_wait
                     and len(ins.sync_info.on_wait) > self.MAX_WAITS]
            if not heavy:
                continue
            extra = {}
            for ins in heavy:
                w = list(ins.sync_info.on_wait)
                ins.sync_info.on_wait = w[:self.MAX_WAITS]
                nops = []
                for i in range(self.MAX_WAITS, len(w), self.MAX_WAITS):
                    n = nc.engines[ins.engine].nop(hint="wait_split")
                    # pop the freshly appended nop from wherever it landed
                    for bb2 in nc.main_func.blocks:
                        if bb2.instructions and bb2.instructions[-1] is n.ins:
                            bb2.instructions.pop()
                            break
                    for wt in w[i:i + self.MAX_WAITS]:
                        handle = _bass_rust.SemaphoreHandle(wt.ant_name, wt.id)
                        _bass_rust.wait_op(n.ins, handle, wt.wait_value,
                                           "sem-ge", False)
                    nops.append(n.ins)
                extra[id(ins)] = nops
            new_list = []
            for ins in bb.instructions:
                new_list.extend(extra.get(id(ins), ()))
                new_list.append(ins)
            bb.instructions[:] = new_list

    def _drain_and_barrier(self, tick_clock, wait_clock):
        nc = self.nc
        self._split_body_waits()
        probe = nc.sync.nop(hint="tile_exit_wait_0")
        wait_clock.add_sem_waits(
            probe.ins, ScopedClock({None: tick_clock.global_clock})
        )
        waits = list(probe.ins.sync_info.on_wait or [])
        probe.ins.sync_info.on_wait = waits[:1]
        for w in waits[1:]:
            n = nc.sync.nop(hint="tile_exit_wait")
            handle = _bass_rust.SemaphoreHandle(w.ant_name, w.id)
            _bass_rust.wait_op(n.ins, handle, w.wait_value, "sem-ge", False)
        nc.sync.drain()
        nc.all_engine_barrier()
        assert self.sems is not None
        popped = nc._tile_sem_poison_stack.pop()
        assert popped is self._sem_poison
        nc.clear_and_free_semaphores(list(self.sems.allocated().values()))
        nc.all_engine_barrier()


def _ap(t, extra, *dims):
    """AP over tile t's buffer: partition dim from the tile, custom free dims.

    dims[0] may override the partition [step, count]."""
    base = t[:]
    return bass.AP(base.tensor, base.offset + extra, list(dims))


def _w0_perm():
    """Row permutation p s.t. w0_permuted[i] = w0[p[i]] matches the kernel's
    feature order: per oc: [S1(9) | S2(81)] then rows 90 + ij*90 + c with
    c<81 -> level4 (ij,kl=c), c>=81 -> level3 (ij, k=c-81)."""
    p = np.empty(OUT_CH * SIGC, np.int64)
    i = 0
    for oc in range(OUT_CH):
        base = oc * SIGC
        p[i:i + 90] = base + np.arange(90)          # S1 then S2, native order
        i += 90
        for ij in range(81):
            # level-4 block (ij, kl) for kl in 0..80 -> orig 819 + ij*81 + kl
            p[i:i + 81] = base + 819 + ij * 81 + np.arange(81)
            i += 81
            # level-3 (ij, k) for k in 0..8 -> orig 90 + ij*9 + k
            p[i:i + 9] = base + 90 + ij * 9 + np.arange(9)
            i += 9
    assert i == OUT_CH * SIGC
    return p


def _build():
    nc = bass.Bass(num_devices=NCORES, target_bir_lowering=True, trn_type="TRN2")

    # ---- per-core DRAM inputs ----
    xs = nc.dram_tensor("xs", [T, BL, IN_CH], F32, kind="ExternalInput")
    cwr = nc.dram_tensor("cwr", [128, 16], F32, kind="ExternalInput")
    cbr = nc.dram_tensor("cbr", [128, OUT_CH], F32, kind="ExternalInput")
    tlin = nc.dram_tensor("tlin", [128, 1], F32, kind="ExternalInput")
    ltri = nc.dram_tensor("ltri", [128, 128], F32R, kind="ExternalInput")
    onef = nc.dram_tensor("onef", [128, 1], F32, kind="ExternalInput")
    oner = nc.dram_tensor("oner", [128, 2], F32R, kind="ExternalInput")
    idn = nc.dram_tensor("idn", [128, 128], F32, kind="ExternalInput")
    w0s = nc.dram_tensor("w0s", [KSH, H0], BF16, kind="ExternalInput")
    w1s = nc.dram_tensor("w1s", [H0, H1], F32, kind="ExternalInput")
    w2s = nc.dram_tensor("w2s", [H1, NCLS], F32, kind="ExternalInput")
    b0c = nc.dram_tensor("b0c", [H0, 1], F32, kind="ExternalInput")
    b1c = nc.dram_tensor("b1c", [H1, 1], F32, kind="ExternalInput")
    b2r = nc.dram_tensor("b2r", [BL, NCLS], F32, kind="ExternalInput")
    out = nc.dram_tensor("out", [BL, NCLS], F32, kind="ExternalOutput")

    with _SplitDrainTileContext(nc) as tc:
        with tc.tile_pool(name="dram", bufs=1, space="DRAM") as dram:
            zl = dram.tile([NCORES * KSH, BL], BF16)     # local z^T, f'-order
            zex = dram.tile([NCORES * KSH, BL], BF16)    # post-A2A shard
            cin = dram.tile([B, H0], F32)
            cout = dram.tile([BL, H0], F32)
            prow = dram.tile([1, LANES * D], F32)        # p[T-1] bounce

            with tc.tile_pool(name="const", bufs=1) as cpool, \
                 tc.tile_pool(name="w0p", bufs=1) as w0pool, \
                 tc.tile_pool(name="prep", bufs=1) as ppool:
                xs_sb = cpool.tile([128, BL * IN_CH], F32)
                nc.scalar.dma_start(xs_sb[:], _ap(xs, 0, [BL * IN_CH, 128],
                                                [1, BL * IN_CH]))
                cw_sb = cpool.tile([128, 16], F32)
                nc.scalar.dma_start(cw_sb[:], cwr[:])
                cb_sb = cpool.tile([128, OUT_CH], F32)
                nc.scalar.dma_start(cb_sb[:], cbr[:])
                tl_sb = cpool.tile([128, 1], F32)
                nc.scalar.dma_start(tl_sb[:], tlin[:])
                lt_sb = cpool.tile([128, 128], F32R)
                nc.scalar.dma_start(lt_sb[:], ltri[:])
                onef_sb = cpool.tile([128, 1], F32)
                nc.scalar.dma_start(onef_sb[:], onef[:])
                oner_sb = cpool.tile([128, 2], F32R)
                nc.scalar.dma_start(oner_sb[:], oner[:])
                idn_sb = cpool.tile([128, 128], F32)
                nc.scalar.dma_start(idn_sb[:], idn[:])
                w1_sb = [cpool.tile([128, H1], F32, tag=f"w1_{j}", name=f"w1_{j}")
                         for j in range(4)]
                for j in range(4):
                    nc.scalar.dma_start(w1_sb[j][:], w1s[j * 128:(j + 1) * 128, :])
                w2_sb = [cpool.tile([128, NCLS], F32, tag=f"w2_{j}", name=f"w2_{j}")
                         for j in range(2)]
                for j in range(2):
                    nc.scalar.dma_start(w2_sb[j][:], w2s[j * 128:(j + 1) * 128, :])
                b0_sb = cpool.tile([128, 4], F32)
                nc.scalar.dma_start(b0_sb[:], _ap(b0c, 0, [1, 128], [128, 4]))
                b1_sb = cpool.tile([128, 2], F32)
                nc.scalar.dma_start(b1_sb[:], _ap(b1c, 0, [1, 128], [128, 2]))
                b2_sb = cpool.tile([BL, NCLS], F32)
                nc.scalar.dma_start(b2_sb[:], b2r[:])
                # ---- prep: conv -> path p, then dx, P1, Y, u, u2, A ----
                W = LANES * D  # 288
                p = ppool.tile([128, W], F32, tag="p")
                dx = ppool.tile([128, W], F32, tag="dx")
                p1 = ppool.tile([128, W], F32, tag="p1")
                yt = ppool.tile([128, W], F32, tag="yt")
                pl = ppool.tile([128, W], F32, tag="pl")
                ut = ppool.tile([128, W], F32, tag="ut")
                u2 = ppool.tile([128, W], F32, tag="u2")
                at = ppool.tile([128, W], F32, tag="at")

                for oc in range(OUT_CH):
                    dst = _ap(p, oc * D + 1, [W, 128], [4 * D, BL], [1, CH])
                    for k in range(4):
                        src = _ap(xs_sb, k, [BL * IN_CH, 128], [IN_CH, BL], [4, CH])
                        wsc = cw_sb[:, oc * 4 + k:oc * 4 + k + 1]
                        if k == 0:
                            nc.vector.tensor_scalar(dst, src, wsc, None, AL.mult)
                        else:
                            nc.vector.scalar_tensor_tensor(
                                dst, src, wsc, dst, AL.mult, AL.add)
                    nc.vector.tensor_scalar(dst, dst,
                                            cb_sb[:, oc:oc + 1], None, AL.add)
                # time channel into col 0 of every lane
                nc.vector.tensor_copy(_ap(p, 0, [W, 128], [D, LANES]),
                                      _ap(tl_sb, 0, [1, 128], [0, LANES]))
                # P1 = p shifted down one step (DMA: compute engines cannot
                # address unaligned partition bases), then dx = p - P1.
                nc.gpsimd.memset(p1[0:1, :], 0.0)
                nc.gpsimd.dma_start(p1[1:128, :], p[0:127, :])
                nc.vector.tensor_tensor(dx[:], p[:], p1[:], AL.subtract)
                # Y[t] = p[T-1] - p[t]  (broadcast last row via DRAM bounce)
                nc.scalar.dma_start(prow[:], p[127:128, :])
                nc.scalar.dma_start(pl[:], _ap(prow, 0, [0, 128], [1, W]))
                nc.vector.tensor_tensor(yt[:], pl[:], p[:], AL.subtract)
                # ut = P1/2 + dx/6, u2 = P1/3 + dx/12, A = P1 + dx/2,
                # dxh = dx/2 (scalars pre-folded so the group outer products
                # are plain TensorTensor ops)
                nc.vector.tensor_scalar(ut[:], p1[:], 0.5, None, AL.mult)
                nc.vector.scalar_tensor_tensor(ut[:], dx[:], 1.0 / 6, ut[:],
                                               AL.mult, AL.add)
                nc.vector.tensor_scalar(u2[:], p1[:], 1.0 / 3, None, AL.mult)
                nc.vector.scalar_tensor_tensor(u2[:], dx[:], 1.0 / 12, u2[:],
                                               AL.mult, AL.add)
                nc.vector.scalar_tensor_tensor(at[:], dx[:], 0.5, p1[:],
                                               AL.mult, AL.add)
                dxh = ppool.tile([128, W], F32, tag="dxh")
                nc.vector.tensor_scalar(dxh[:], dx[:], 0.5, None, AL.mult)
                # preload ACT function tables off the critical tail
                dum = ppool.tile([1, 4], F32, tag="dum")
                nc.scalar.activation(dum[0:1, 0:1], tl_sb[0:1, 0:1], AF.Sigmoid)
                nc.scalar.activation(dum[0:1, 1:2], tl_sb[0:1, 0:1], AF.Exp)
                nc.scalar.activation(dum[0:1, 2:3], onef_sb[0:1, 0:1], AF.Ln)

                # ---- per-lane feature tiles, filled group by group ----
                zt4all = cpool.tile([81, OUT_CH * 720], BF16, tag="zt4all",
                                    name="zt4all")
                z1all = cpool.tile([9, LANES], BF16, tag="z1all", name="z1all")
                z2all = cpool.tile([81, LANES], BF16, tag="z2all", name="z2all")

                with tc.tile_pool(name="grp", bufs=2) as gpool, \
                     tc.tile_pool(name="ps2", bufs=2, space="PSUM") as ps2, \
                     tc.tile_pool(name="ptab", bufs=1, space="PSUM") as ptab, \
                     tc.tile_pool(name="ps12", bufs=1, space="PSUM") as ps12:
                    for g in range(NG):
                        off = g * GL * D  # col offset into the 288-wide tiles
                        GW = GL * 81      # 648

                        def o_ij(t, st=1):  # [lane, i(step), j(bcast)] view
                            return _ap(t, off, [W, 128], [D, GL], [st, D], [0, D])

                        def o_ji(t, st=1):  # [lane, i(bcast), j(step)] view
                            return _ap(t, off, [W, 128], [D, GL], [0, D], [st, D])

                        m2 = gpool.tile([128, GW], F32R, tag="m2")
                        m2v = _ap(m2, 0, [GW, 128], [81, GL], [D, D], [1, D])
                        nc.vector.tensor_tensor(m2v, o_ij(at), o_ji(dx), AL.mult)


                        s2 = ps2.tile([128, GW], F32, tag="s2")
                        nc.tensor.matmul(s2[:, 0:512], lt_sb[:], m2[:, 0:512],
                                         start=True, stop=True)
                        nc.tensor.matmul(s2[:, 512:GW], lt_sb[:], m2[:, 512:GW],
                                         start=True, stop=True)

                        bt = gpool.tile([128, GW], F32R, tag="bt")
                        btv = _ap(bt, 0, [GW, 128], [81, GL], [D, D], [1, D])
                        nc.vector.tensor_tensor(btv, o_ij(ut), o_ji(dx), AL.mult)
                        nc.vector.tensor_tensor(bt[:], bt[:], s2[:], AL.add)

                        t8 = gpool.tile([128, GW], F32R, tag="t8")
                        t8v = _ap(t8, 0, [GW, 128], [81, GL], [D, D], [1, D])
                        nc.vector.tensor_tensor(t8v, o_ij(u2), o_ji(dx), AL.mult)
                        nc.vector.tensor_tensor(t8[:], t8[:], s2[:], AL.add)

                        # 82-wide lane stride: f32r matmuls need even N,
                        # so the mmTB rhs is [128, 82] with a zero last column
                        q2 = gpool.tile([128, GL * 82], F32R, tag="q2")
                        q2v = _ap(q2, 0, [GL * 82, 128], [82, GL], [D, D], [1, D])
                        nc.vector.tensor_tensor(q2v, o_ij(dxh), o_ji(dx), AL.mult)
                        nc.vector.tensor_scalar(
                            _ap(q2, 81, [GL * 82, 128], [82, GL]),
                            _ap(dxh, 0, [LANES * D, 128], [0, GL]),
                            0.0, None, AL.mult)

                        rx = gpool.tile([128, GL * 90], F32R, tag="rx")
                        rxv = _ap(rx, 0, [GL * 90, 128], [90, GL], [D, D], [1, D])
                        nc.vector.tensor_tensor(rxv, o_ij(dx), o_ji(yt), AL.mult)
                        nc.vector.tensor_copy(
                            _ap(rx, 81, [GL * 90, 128], [90, GL], [1, D]),
                            _ap(dx, off, [W, 128], [D, GL], [1, D]))

                        tab = ptab.tile([128, 1024], F32, tag="tab")
                        s12 = ps12.tile([128, 4 * GL], F32, tag="s12")
                        for l in range(GL):
                            lane = g * GL + l
                            bloc, oc = lane // OUT_CH, lane % OUT_CH
                            nc.tensor.matmul(
                                _ap(tab, 128 * l, [1024, 81], [1, 90]),
                                bt[:, l * 81:(l + 1) * 81],
                                rx[:, l * 90:(l + 1) * 90],
                                start=True, stop=False)
                            nc.tensor.matmul(
                                _ap(tab, 128 * l, [1024, 81], [1, 82]),
                                t8[:, l * 81:(l + 1) * 81],
                                q2[:, l * 82:(l + 1) * 82],
                                start=False, stop=True)
                            nc.tensor.matmul(s12[0:9, 4 * l:4 * l + 1],
                                             dx[:, lane * D:(lane + 1) * D],
                                             onef_sb[:], start=True, stop=True)
                            nc.tensor.matmul(s12[0:81, 4 * l + 2:4 * l + 4],
                                             m2[:, l * 81:(l + 1) * 81],
                                             oner_sb[:], start=True, stop=True)
                        # batched PSUM->SBUF evacuation for the whole group:
                        # lanes l = bo*4 + oc, local batch bloc = 2g + bo
                        nc.vector.tensor_copy(
                            _ap(zt4all, 2 * g, [2880, 81],
                                [720, OUT_CH], [8, 90]),
                            _ap(tab, 0, [1024, 81],
                                [128, OUT_CH], [1, 90]))
                        nc.scalar.activation(
                            _ap(zt4all, 2 * g + 1, [2880, 81],
                                [720, OUT_CH], [8, 90]),
                            _ap(tab, 512, [1024, 81],
                                [128, OUT_CH], [1, 90]),
                            AF.Copy)
                        nc.scalar.activation(
                            _ap(z1all, 2 * g, [LANES, 9], [1, 2], [8, OUT_CH]),
                            _ap(s12, 0, [4 * GL, 9], [16, 2], [4, OUT_CH]),
                            AF.Copy)
                        nc.scalar.activation(
                            _ap(z2all, 2 * g, [LANES, 81], [1, 2], [8, OUT_CH]),
                            _ap(s12, 2, [4 * GL, 81], [16, 2], [4, OUT_CH]),
                            AF.Copy)

                # ---- pack z^T shard-ordered into DRAM and exchange ----
                for oc in range(OUT_CH):
                    base = oc * SIGC * BL
                    eng = nc.scalar if oc < 2 else nc.sync
                    eng.dma_start(
                        _ap(zl, base, [8, 9], [1, 8]),
                        z1all[:, oc * BL:(oc + 1) * BL])
                    eng.dma_start(
                        _ap(zl, base + 9 * BL, [8, 81], [1, 8]),
                        z2all[:, oc * BL:(oc + 1) * BL])
                    eng.dma_start(
                        _ap(zl, base + 90 * BL, [720, 81], [8, 90], [1, 8]),
                        zt4all[:, oc * 720:(oc + 1) * 720])
                # w0 shard prefetch (emitted here so these no-dep DMAs sit
                # behind nothing critical on the sync queue, and stream
                # during the signature phase / AllToAll)
                w0_t = []
                for i in range(29):
                    k0 = i * 128
                    kn = min(128, KSH - k0)
                    wt = w0pool.tile([128, H0], BF16, tag=f"w0_{i}", name=f"w0_{i}")
                    nc.sync.dma_start(wt[0:kn, :], w0s[k0:k0 + kn, :])
                    w0_t.append((wt, k0, kn))
                nc.gpsimd.collective_compute(
                    "AllToAll", AL.bypass,
                    replica_groups=[list(range(NCORES))],
                    ins=[zl[:].opt()], outs=[zex[:].opt()])

                # ---- z0 = z @ w0 partial over this core's K shard ----
                with tc.tile_pool(name="zex", bufs=8) as zpool, \
                     tc.tile_pool(name="pz0", bufs=1, space="PSUM") as pz0p, \
                     tc.tile_pool(name="ptail", bufs=1, space="PSUM") as ptail:
                    z0p = pz0p.tile([B, H0], F32, tag="z0p")
                    for i, (wt, k0, kn) in enumerate(w0_t):
                        ztl = zpool.tile([128, B], BF16, tag="ztl")
                        eng = nc.scalar if i % 2 else nc.sync
                        eng.dma_start(
                            ztl[0:kn, :],
                            _ap(zex, k0 * BL, [BL, kn],
                                [KSH * BL, NCORES], [1, BL]))
                        nc.tensor.matmul(z0p[:], ztl[0:kn, :], wt[0:kn, :],
                                         start=(i == 0), stop=(i == 28))
                    z0sb = cpool.tile([B, H0], F32, tag="z0sb", name="z0sb")
                    nc.vector.tensor_copy(z0sb[:], z0p[:])
                    nc.gpsimd.dma_start(cin[:], z0sb[:])
                    nc.gpsimd.collective_compute(
                        "ReduceScatter", AL.add,
                        replica_groups=[list(range(NCORES))],
                        ins=[cin[:].opt()], outs=[cout[:].opt()])

                    # ---- tail: sigmoid(w0 out) -> w1 -> sigmoid -> w2 ----
                    z1row = cpool.tile([BL, H0], F32, tag="z1row")
                    nc.gpsimd.dma_start(z1row[:], cout[:])
                    pz1 = ptail.tile([128, 4 * BL], F32, tag="pz1")
                    z1t = cpool.tile([128, 4 * BL], F32, tag="z1t")
                    for j in range(4):
                        nc.tensor.transpose(pz1[:, j * BL:(j + 1) * BL],
                                            z1row[:, j * 128:(j + 1) * 128],
                                            idn_sb[0:BL, 0:BL])
                        nc.scalar.activation(z1t[:, j * BL:(j + 1) * BL],
                                             pz1[:, j * BL:(j + 1) * BL],
                                             AF.Sigmoid, bias=b0_sb[:, j:j + 1])
                    pz2 = ptail.tile([128, 2 * BL], F32, tag="pz2")
                    z2t = cpool.tile([128, 2 * BL], F32, tag="z2t")
                    for m in range(2):
                        for kj in range(4):
                            nc.tensor.matmul(
                                pz2[:, m * BL:(m + 1) * BL],
                                w1_sb[kj][:, m * 128:(m + 1) * 128],
                                z1t[:, kj * BL:(kj + 1) * BL],
                                start=(kj == 0), stop=(kj == 3))
                        nc.scalar.activation(z2t[:, m * BL:(m + 1) * BL],
                                             pz2[:, m * BL:(m + 1) * BL],
                                             AF.Sigmoid, bias=b1_sb[:, m:m + 1])
                    pz3 = ptail.tile([BL, NCLS], F32, tag="pz3")
                    for m in range(2):
                        nc.tensor.matmul(pz3[:], z2t[:, m * BL:(m + 1) * BL],
                                         w2_sb[m][:], start=(m == 0),
                                         stop=(m == 1))
                    z3 = cpool.tile([BL, NCLS], F32, tag="z3")
                    nc.vector.tensor_tensor(z3[:], pz3[:], b2_sb[:], AL.add)
                    mx = cpool.tile([BL, 1], F32, tag="mx")
                    nc.vector.tensor_reduce(mx[:], z3[:], mybir.AxisListType.X,
                                            AL.max)
                    tm = cpool.tile([BL, NCLS], F32, tag="tm")
                    nc.vector.tensor_scalar(tm[:], z3[:], mx[:, 0:1], None,
                                            AL.subtract)
                    ex = cpool.tile([BL, NCLS], F32, tag="ex")
                    se = cpool.tile([BL, 1], F32, tag="se")
                    nc.scalar.activation(ex[:], tm[:], AF.Exp, accum_out=se[:])
                    ls = cpool.tile([BL, 1], F32, tag="ls")
                    nc.scalar.activation(ls[:], se[:], AF.Ln)
                    osb = cpool.tile([BL, NCLS], F32, tag="osb")
                    nc.vector.tensor_scalar(osb[:], tm[:], ls[:, 0:1], None,
                                            AL.subtract)
                    nc.gpsimd.dma_start(out[:], osb[:])
    return nc


_CACHE = {}


def kernel(x, conv_w, conv_b, w0, b0, w1, b1, w2, b2):
    x = np.ascontiguousarray(np.asarray(x, np.float32))
    conv_w = np.asarray(conv_w, np.float32)
    conv_b = np.asarray(conv_b, np.float32)
    w0 = np.asarray(w0, np.float32)
    w1 = np.ascontiguousarray(np.asarray(w1, np.float32))
    w2 = np.ascontiguousarray(np.asarray(w2, np.float32))
    b0 = np.asarray(b0, np.float32)
    b1 = np.asarray(b1, np.float32)
    b2 = np.asarray(b2, np.float32)

    if "nc" not in _CACHE:
        _CACHE["nc"] = _build()
        _CACHE["perm"] = _w0_perm()
    nc = _CACHE["nc"]
    bf16 = mybir.dt.np(BF16)
    w0p = np.ascontiguousarray(w0[_CACHE["perm"], :]).astype(bf16)

    shared = {
        "cwr": np.ascontiguousarray(
            np.broadcast_to(conv_w.reshape(1, 16), (128, 16))),
        "cbr": np.ascontiguousarray(
            np.broadcast_to(conv_b.reshape(1, OUT_CH), (128, OUT_CH))),
        "tlin": np.linspace(0.0, 1.0, T, dtype=np.float32).reshape(128, 1),
        "ltri": np.ascontiguousarray(
            np.triu(np.ones((128, 128), np.float32), 1)),
        "onef": np.ones((128, 1), np.float32),
        "oner": np.ones((128, 2), np.float32),
        "idn": np.eye(128, dtype=np.float32),
        "w1s": w1, "w2s": w2,
        "b0c": b0.reshape(H0, 1), "b1c": b1.reshape(H1, 1),
        "b2r": np.ascontiguousarray(np.broadcast_to(b2.reshape(1, NCLS),
                                                    (BL, NCLS))),
    }
    in_maps = []
    for c in range(NCORES):
        m = dict(shared)
        m["xs"] = np.ascontiguousarray(
            x[c * BL:(c + 1) * BL, 0].transpose(1, 0, 2))
        m["w0s"] = np.ascontiguousarray(w0p[c * KSH:(c + 1) * KSH, :])
        in_maps.append(m)

    _CACHE["in_maps"] = in_maps
    res = run_bass_kernel_spmd(nc, in_maps, core_ids=list(range(NCORES)))
    return np.concatenate([res.results[c]["out"] for c in range(NCORES)],
                          axis=0)


# revision 32
# speedup vs baseline: 1.3790x; 1.0274x over previous
"""CNN + truncated path-signature (depth 4) + FF head on 8 TRN2 NeuronCores.

Strategy
--------
- Batch data-parallel signature computation: core c handles batches
  [8c, 8c+8) = 32 (batch, out_ch) lanes, T=128 time steps on partitions.
- Signature reformulated to avoid sequential scans:
    dx, P1 (= shifted path), Y (= suffix sums) are free;
    the only prefix scan (level 2, S2pre) is one triangular matmul L @ M2;
    S3 = B^T X;  S4 = B^T R + tmp81'^T (dx(x)dx)/2   (suffix-vector trick),
  so levels 3 and 4 are plain T-contractions on the TensorEngine.
- The 60 MB w0 is row-sharded 8 ways (this is the memory roofline win):
  AllToAll re-shards the signature activations feature-wise, each core
  multiplies its [3690, 512] w0 shard, ReduceScatter returns each core its
  own 8 batches, and the small w1/w2 head finishes per-core.
- w0 rows are permuted host-side to match the kernel's natural feature
  order, so no on-device transposes are needed.
"""
import os
import sys
sys.path.insert(0, "/opt/trn_rl_repo")
if os.environ.get("JAX_PLATFORMS") == "cpu":
    # The SPMD launch needs the axon/neuron PJRT backend.
    os.environ["JAX_PLATFORMS"] = ""

import numpy as np
import bass_rust as _bass_rust
import concourse.bass as bass
import concourse.tile as tile
import concourse.mybir as mybir
from concourse.vector_clock import ScopedClock
from concourse.bass_utils import run_bass_kernel_spmd

F32 = mybir.dt.float32
F32R = mybir.dt.float32r
BF16 = mybir.dt.bfloat16
AL = mybir.AluOpType
AF = mybir.ActivationFunctionType

NCORES = 8
B, T, IN_CH = 64, 128, 32
OUT_CH, CH, D = 4, 8, 9            # conv out-channels, conv width, path dim
BL = B // NCORES                   # local batches = 8
LANES = BL * OUT_CH                # 32 lanes/core
NG = 4                             # lane groups of 8
GL = 8                             # lanes per group
SIGC = 7380                        # per-lane signature channels
KSH = OUT_CH * SIGC // NCORES      # w0 K-shard rows per core = 3690
H0, H1, NCLS = 512, 256, 10


class _SplitDrainTileContext(tile.TileContext):
    """Tile exit drain carries one sem wait per CTRL instruction.

    This container's walrus build rejects >2 sync waits on a CTRL
    instruction; Tile's exit drain waits on the whole global clock.
    Redistribute the waits over nops on the same engine (program order on
    one engine preserves semantics)."""

    MAX_WAITS = 1

    def _split_body_waits(self):
        """Move excess sem waits from any instruction onto preceding nops on
        the same engine (same-engine program order preserves semantics)."""
        nc = self.nc
        for bb in nc.main_func.blocks:
            heavy = [ins for ins in bb.instructions
                     if ins.sync_info and ins.sync_info.on_wait
                     and len(ins.sync_info.on_wait) > self.MAX_WAITS]
            if not heavy:
                continue
            extra = {}
            for ins in heavy:
                w = list(ins.sync_info.on_wait)
                ins.sync_info.on_wait = w[:self.MAX_WAITS]
                nops = []
                for i in range(self.MAX_WAITS, len(w), self.MAX_WAITS):
                    n = nc.engines[ins.engine].nop(hint="wait_split")
                    # pop the freshly appended nop from wherever it landed
                    for bb2 in nc.main_func.blocks:
                        if bb2.instructions and bb2.instructions[-1] is n.ins:
                            bb2.instructions.pop()
                            break
                    for wt in w[i:i + self.MAX_WAITS]:
                        handle = _bass_rust.SemaphoreHandle(wt.ant_name, wt.id)
                        _bass_rust.wait_op(n.ins, handle, wt.wait_value,
                                           "sem-ge", False)
                    nops.append(n.ins)
                extra[id(ins)] = nops
            new_list = []
            for ins in bb.instructions:
                new_list.extend(extra.get(id(ins), ()))
                new_list.append(ins)
            bb.instructions[:] = new_list

    def _drain_and_barrier(self, tick_clock, wait_clock):
        nc = self.nc
        self._split_body_waits()
        probe = nc.sync.nop(hint="tile_exit_wait_0")
        wait_clock.add_sem_waits(
            probe.ins, ScopedClock({None: tick_clock.global_clock})
        )
        waits = list(probe.ins.sync_info.on_wait or [])
        probe.ins.sync_info.on_wait = waits[:1]
        for w in waits[1:]:
            n = nc.sync.nop(hint="tile_exit_wait")
            handle = _bass_rust.SemaphoreHandle(w.ant_name, w.id)
            _bass_rust.wait_op(n.ins, handle, w.wait_value, "sem-ge", False)
        nc.sync.drain()
        nc.all_engine_barrier()
        assert self.sems is not None
        popped = nc._tile_sem_poison_stack.pop()
        assert popped is self._sem_poison
        nc.clear_and_free_semaphores(list(self.sems.allocated().values()))
        nc.all_engine_barrier()


def _ap(t, extra, *dims):
    """AP over tile t's buffer: partition dim from the tile, custom free dims.

    dims[0] may override the partition [step, count]."""
    base = t[:]
    return bass.AP(base.tensor, base.offset + extra, list(dims))


def _w0_perm():
    """Row permutation p s.t. w0_permuted[i] = w0[p[i]] matches the kernel's
    feature order: per oc: [S1(9) | S2(81)] then rows 90 + ij*90 + c with
    c<81 -> level4 (ij,kl=c), c>=81 -> level3 (ij, k=c-81)."""
    p = np.empty(OUT_CH * SIGC, np.int64)
    i = 0
    for oc in range(OUT_CH):
        base = oc * SIGC
        p[i:i + 90] = base + np.arange(90)          # S1 then S2, native order
        i += 90
        for ij in range(81):
            # level-4 block (ij, kl) for kl in 0..80 -> orig 819 + ij*81 + kl
            p[i:i + 81] = base + 819 + ij * 81 + np.arange(81)
            i += 81
            # level-3 (ij, k) for k in 0..8 -> orig 90 + ij*9 + k
            p[i:i + 9] = base + 90 + ij * 9 + np.arange(9)
            i += 9
    assert i == OUT_CH * SIGC
    return p


def _build():
    nc = bass.Bass(num_devices=NCORES, target_bir_lowering=True, trn_type="TRN2")

    # ---- per-core DRAM inputs ----
    xs = nc.dram_tensor("xs", [T, BL, IN_CH], F32, kind="ExternalInput")
    cwr = nc.dram_tensor("cwr", [128, 16], F32, kind="ExternalInput")
    cbr = nc.dram_tensor("cbr", [128, OUT_CH], F32, kind="ExternalInput")
    tlin = nc.dram_tensor("tlin", [128, 1], F32, kind="ExternalInput")
    ltri = nc.dram_tensor("ltri", [128, 128], F32R, kind="ExternalInput")
    onef = nc.dram_tensor("onef", [128, 1], F32, kind="ExternalInput")
    oner = nc.dram_tensor("oner", [128, 2], F32R, kind="ExternalInput")
    idn = nc.dram_tensor("idn", [128, 128], F32, kind="ExternalInput")
    w0s = nc.dram_tensor("w0s", [KSH, H0], BF16, kind="ExternalInput")
    w1s = nc.dram_tensor("w1s", [H0, H1], F32, kind="ExternalInput")
    w2s = nc.dram_tensor("w2s", [H1, NCLS], F32, kind="ExternalInput")
    b0c = nc.dram_tensor("b0c", [H0, 1], F32, kind="ExternalInput")
    b1c = nc.dram_tensor("b1c", [H1, 1], F32, kind="ExternalInput")
    b2r = nc.dram_tensor("b2r", [BL, NCLS], F32, kind="ExternalInput")
    out = nc.dram_tensor("out", [BL, NCLS], F32, kind="ExternalOutput")

    with _SplitDrainTileContext(nc) as tc:
        with tc.tile_pool(name="dram", bufs=1, space="DRAM") as dram:
            zl = dram.tile([NCORES * KSH, BL], BF16)     # local z^T, f'-order
            zex = dram.tile([NCORES * KSH, BL], BF16)    # post-A2A shard
            cin = dram.tile([B, H0], F32)
            cout = dram.tile([BL, H0], F32)
            prow = dram.tile([1, LANES * D], F32)        # p[T-1] bounce

            with tc.tile_pool(name="const", bufs=1) as cpool, \
                 tc.tile_pool(name="w0p", bufs=1) as w0pool, \
                 tc.tile_pool(name="prep", bufs=1) as ppool:
                xs_sb = cpool.tile([128, BL * IN_CH], F32)
                nc.scalar.dma_start(xs_sb[:], _ap(xs, 0, [BL * IN_CH, 128],
                                                [1, BL * IN_CH]))
                cw_sb = cpool.tile([128, 16], F32)
                nc.scalar.dma_start(cw_sb[:], cwr[:])
                cb_sb = cpool.tile([128, OUT_CH], F32)
                nc.scalar.dma_start(cb_sb[:], cbr[:])
                tl_sb = cpool.tile([128, 1], F32)
                nc.scalar.dma_start(tl_sb[:], tlin[:])
                lt_sb = cpool.tile([128, 128], F32R)
                nc.scalar.dma_start(lt_sb[:], ltri[:])
                onef_sb = cpool.tile([128, 1], F32)
                nc.scalar.dma_start(onef_sb[:], onef[:])
                oner_sb = cpool.tile([128, 2], F32R)
                nc.scalar.dma_start(oner_sb[:], oner[:])
                idn_sb = cpool.tile([128, 128], F32)
                nc.scalar.dma_start(idn_sb[:], idn[:])
                w1_sb = [cpool.tile([128, H1], F32, tag=f"w1_{j}", name=f"w1_{j}")
                         for j in range(4)]
                for j in range(4):
                    nc.scalar.dma_start(w1_sb[j][:], w1s[j * 128:(j + 1) * 128, :])
                w2_sb = [cpool.tile([128, NCLS], F32, tag=f"w2_{j}", name=f"w2_{j}")
                         for j in range(2)]
                for j in range(2):
                    nc.scalar.dma_start(w2_sb[j][:], w2s[j * 128:(j + 1) * 128, :])
                b0_sb = cpool.tile([128, 4], F32)
                nc.scalar.dma_start(b0_sb[:], _ap(b0c, 0, [1, 128], [128, 4]))
                b1_sb = cpool.tile([128, 2], F32)
                nc.scalar.dma_start(b1_sb[:], _ap(b1c, 0, [1, 128], [128, 2]))
                b2_sb = cpool.tile([BL, NCLS], F32)
                nc.scalar.dma_start(b2_sb[:], b2r[:])
                # ---- prep: conv -> path p, then dx, P1, Y, u, u2, A ----
                W = LANES * D  # 288
                p = ppool.tile([128, W], F32, tag="p")
                dx = ppool.tile([128, W], F32, tag="dx")
                p1 = ppool.tile([128, W], F32, tag="p1")
                yt = ppool.tile([128, W], F32, tag="yt")
                pl = ppool.tile([128, W], F32, tag="pl")
                ut = ppool.tile([128, W], F32, tag="ut")
                u2 = ppool.tile([128, W], F32, tag="u2")
                at = ppool.tile([128, W], F32, tag="at")

                # conv, batched over (oc, b, ch): p_data = sum_k x_k * w[oc,k]
                pdst = _ap(p, 1, [W, 128], [D, OUT_CH], [4 * D, BL], [1, CH])
                tmpc = ppool.tile([128, BL * IN_CH], F32, tag="tmpc")
                tview = _ap(tmpc, 0, [BL * IN_CH, 128],
                            [CH * BL, OUT_CH], [CH, BL], [1, CH])
                for k in range(4):
                    xsv = _ap(xs_sb, k, [BL * IN_CH, 128],
                              [0, OUT_CH], [IN_CH, BL], [4, CH])
                    cwv = _ap(cw_sb, k, [16, 128], [4, OUT_CH], [0, BL], [0, CH])
                    if k == 0:
                        nc.vector.tensor_tensor(pdst, xsv, cwv, AL.mult)
                    else:
                        nc.vector.tensor_tensor(tview, xsv, cwv, AL.mult)
                        nc.vector.tensor_tensor(pdst, pdst, tview, AL.add)
                cbv = _ap(cb_sb, 0, [OUT_CH, 128], [1, OUT_CH], [0, BL], [0, CH])
                nc.vector.tensor_tensor(pdst, pdst, cbv, AL.add)
                # time channel into col 0 of every lane
                nc.vector.tensor_copy(_ap(p, 0, [W, 128], [D, LANES]),
                                      _ap(tl_sb, 0, [1, 128], [0, LANES]))
                # P1 = p shifted down one step (DMA: compute engines cannot
                # address unaligned partition bases), then dx = p - P1.
                nc.gpsimd.memset(p1[0:1, :], 0.0)
                nc.gpsimd.dma_start(p1[1:128, :], p[0:127, :])
                nc.vector.tensor_tensor(dx[:], p[:], p1[:], AL.subtract)
                # Y[t] = p[T-1] - p[t]  (broadcast last row via DRAM bounce)
                nc.scalar.dma_start(prow[:], p[127:128, :])
                nc.scalar.dma_start(pl[:], _ap(prow, 0, [0, 128], [1, W]))
                # S1 = p[T-1] per lane: gather straight out of the bounce row
                z1tmp = ppool.tile([9, LANES], F32, tag="z1tmp")
                nc.scalar.dma_start(
                    z1tmp[:], _ap(prow, 0, [1, 9], [D, LANES]))
                nc.vector.tensor_tensor(yt[:], pl[:], p[:], AL.subtract)
                # ut = P1/2 + dx/6, u2 = P1/3 + dx/12, A = P1 + dx/2,
                # dxh = dx/2 (scalars pre-folded so the group outer products
                # are plain TensorTensor ops)
                nc.vector.tensor_scalar(ut[:], p1[:], 0.5, None, AL.mult)
                nc.vector.scalar_tensor_tensor(ut[:], dx[:], 1.0 / 6, ut[:],
                                               AL.mult, AL.add)
                nc.vector.tensor_scalar(u2[:], p1[:], 1.0 / 3, None, AL.mult)
                nc.vector.scalar_tensor_tensor(u2[:], dx[:], 1.0 / 12, u2[:],
                                               AL.mult, AL.add)
                nc.vector.scalar_tensor_tensor(at[:], dx[:], 0.5, p1[:],
                                               AL.mult, AL.add)
                dxh = ppool.tile([128, W], F32, tag="dxh")
                nc.vector.tensor_scalar(dxh[:], dx[:], 0.5, None, AL.mult)
                # preload ACT function tables off the critical tail
                dum = ppool.tile([1, 4], F32, tag="dum")
                nc.scalar.activation(dum[0:1, 0:1], tl_sb[0:1, 0:1], AF.Sigmoid)
                nc.scalar.activation(dum[0:1, 1:2], tl_sb[0:1, 0:1], AF.Exp)
                nc.scalar.activation(dum[0:1, 2:3], onef_sb[0:1, 0:1], AF.Ln)

                # ---- per-lane feature tiles, filled group by group ----
                zt4all = cpool.tile([81, OUT_CH * 720], BF16, tag="zt4all",
                                    name="zt4all")
                z1all = cpool.tile([9, LANES], BF16, tag="z1all", name="z1all")
                z2all = cpool.tile([81, LANES], BF16, tag="z2all", name="z2all")

                with tc.tile_pool(name="grp", bufs=2) as gpool, \
                     tc.tile_pool(name="ps2", bufs=2, space="PSUM") as ps2, \
                     tc.tile_pool(name="ptab", bufs=1, space="PSUM") as ptab, \
                     tc.tile_pool(name="ps12", bufs=1, space="PSUM") as ps12:
                    for g in range(NG):
                        off = g * GL * D  # col offset into the 288-wide tiles
                        GW = GL * 81      # 648

                        def o_ij(t, st=1):  # [lane, i(step), j(bcast)] view
                            return _ap(t, off, [W, 128], [D, GL], [st, D], [0, D])

                        def o_ji(t, st=1):  # [lane, i(bcast), j(step)] view
                            return _ap(t, off, [W, 128], [D, GL], [0, D], [st, D])

                        m2 = gpool.tile([128, GW], F32R, tag="m2")
                        m2v = _ap(m2, 0, [GW, 128], [81, GL], [D, D], [1, D])
                        nc.vector.tensor_tensor(m2v, o_ij(at), o_ji(dx), AL.mult)


                        s2 = ps2.tile([128, GW], F32, tag="s2")
                        nc.tensor.matmul(s2[:, 0:512], lt_sb[:], m2[:, 0:512],
                                         start=True, stop=True)
                        nc.tensor.matmul(s2[:, 512:GW], lt_sb[:], m2[:, 512:GW],
                                         start=True, stop=True)

                        bt = gpool.tile([128, GW], F32R, tag="bt")
                        btv = _ap(bt, 0, [GW, 128], [81, GL], [D, D], [1, D])
                        nc.vector.tensor_tensor(btv, o_ij(ut), o_ji(dx), AL.mult)
                        nc.vector.tensor_tensor(bt[:], bt[:], s2[:], AL.add)

                        t8 = gpool.tile([128, GW], F32R, tag="t8")
                        t8v = _ap(t8, 0, [GW, 128], [81, GL], [D, D], [1, D])
                        nc.vector.tensor_tensor(t8v, o_ij(u2), o_ji(dx), AL.mult)
                        nc.vector.tensor_tensor(t8[:], t8[:], s2[:], AL.add)

                        # 82-wide lane stride: f32r matmuls need even N,
                        # so the mmTB rhs is [128, 82] with a zero last column
                        q2 = gpool.tile([128, GL * 82], F32R, tag="q2")
                        q2v = _ap(q2, 0, [GL * 82, 128], [82, GL], [D, D], [1, D])
                        nc.vector.tensor_tensor(q2v, o_ij(dxh), o_ji(dx), AL.mult)
                        nc.vector.tensor_scalar(
                            _ap(q2, 81, [GL * 82, 128], [82, GL]),
                            _ap(dxh, 0, [LANES * D, 128], [0, GL]),
                            0.0, None, AL.mult)

                        rx = gpool.tile([128, GL * 90], F32R, tag="rx")
                        rxv = _ap(rx, 0, [GL * 90, 128], [90, GL], [D, D], [1, D])
                        nc.vector.tensor_tensor(rxv, o_ij(dx), o_ji(yt), AL.mult)
                        nc.vector.tensor_copy(
                            _ap(rx, 81, [GL * 90, 128], [90, GL], [1, D]),
                            _ap(dx, off, [W, 128], [D, GL], [1, D]))

                        tab = ptab.tile([128, 1024], F32, tag="tab")
                        s12 = ps12.tile([128, 2 * GL], F32, tag="s12")
                        for l in range(GL):
                            lane = g * GL + l
                            bloc, oc = lane // OUT_CH, lane % OUT_CH
                            nc.tensor.matmul(
                                _ap(tab, 128 * l, [1024, 81], [1, 90]),
                                bt[:, l * 81:(l + 1) * 81],
                                rx[:, l * 90:(l + 1) * 90],
                                start=True, stop=False)
                            nc.tensor.matmul(
                                _ap(tab, 128 * l, [1024, 81], [1, 82]),
                                t8[:, l * 81:(l + 1) * 81],
                                q2[:, l * 82:(l + 1) * 82],
                                start=False, stop=True)
                            nc.tensor.matmul(s12[0:81, 2 * l:2 * l + 2],
                                             m2[:, l * 81:(l + 1) * 81],
                                             oner_sb[:], start=True, stop=True)
                        # batched PSUM->SBUF evacuation for the whole group:
                        # lanes l = bo*4 + oc, local batch bloc = 2g + bo
                        nc.vector.tensor_copy(
                            _ap(zt4all, 2 * g, [2880, 81],
                                [720, OUT_CH], [8, 90]),
                            _ap(tab, 0, [1024, 81],
                                [128, OUT_CH], [1, 90]))
                        nc.scalar.activation(
                            _ap(zt4all, 2 * g + 1, [2880, 81],
                                [720, OUT_CH], [8, 90]),
                            _ap(tab, 512, [1024, 81],
                                [128, OUT_CH], [1, 90]),
                            AF.Copy)
                        nc.scalar.activation(
                            _ap(z2all, 2 * g, [LANES, 81], [1, 2], [8, OUT_CH]),
                            _ap(s12, 0, [2 * GL, 81], [8, 2], [2, OUT_CH]),
                            AF.Copy)

                # ---- pack z^T shard-ordered into DRAM and exchange ----
                nc.scalar.activation(
                    _ap(z1all, 0, [LANES, 9], [8, OUT_CH], [1, BL]),
                    _ap(z1tmp, 0, [LANES, 9], [1, OUT_CH], [OUT_CH, BL]),
                    AF.Copy)
                for oc in range(OUT_CH):
                    base = oc * SIGC * BL
                    eng = nc.scalar if oc < 2 else nc.sync
                    eng.dma_start(
                        _ap(zl, base, [8, 9], [1, 8]),
                        z1all[:, oc * BL:(oc + 1) * BL])
                    eng.dma_start(
                        _ap(zl, base + 9 * BL, [8, 81], [1, 8]),
                        z2all[:, oc * BL:(oc + 1) * BL])
                    eng.dma_start(
                        _ap(zl, base + 90 * BL, [720, 81], [8, 90], [1, 8]),
                        zt4all[:, oc * 720:(oc + 1) * 720])
                # w0 shard prefetch (emitted here so these no-dep DMAs sit
                # behind nothing critical on the sync queue, and stream
                # during the signature phase / AllToAll)
                w0_t = []
                for i in range(29):
                    k0 = i * 128
                    kn = min(128, KSH - k0)
                    wt = w0pool.tile([128, H0], BF16, tag=f"w0_{i}", name=f"w0_{i}")
                    nc.sync.dma_start(wt[0:kn, :], w0s[k0:k0 + kn, :])
                    w0_t.append((wt, k0, kn))
                nc.gpsimd.collective_compute(
                    "AllToAll", AL.bypass,
                    replica_groups=[list(range(NCORES))],
                    ins=[zl[:].opt()], outs=[zex[:].opt()])

                # ---- z0 = z @ w0 partial over this core's K shard ----
                with tc.tile_pool(name="zex", bufs=8) as zpool, \
                     tc.tile_pool(name="pz0", bufs=1, space="PSUM") as pz0p, \
                     tc.tile_pool(name="ptail", bufs=1, space="PSUM") as ptail:
                    z0p = pz0p.tile([B, H0], F32, tag="z0p")
                    for i, (wt, k0, kn) in enumerate(w0_t):
                        ztl = zpool.tile([128, B], BF16, tag="ztl")
                        eng = (nc.sync, nc.scalar, nc.gpsimd)[i % 3]
                        eng.dma_start(
                            ztl[0:kn, :],
                            _ap(zex, k0 * BL, [BL, kn],
                                [KSH * BL, NCORES], [1, BL]))
                        nc.tensor.matmul(z0p[:], ztl[0:kn, :], wt[0:kn, :],
                                         start=(i == 0), stop=(i == 28))
                    z0sb = cpool.tile([B, H0], F32, tag="z0sb", name="z0sb")
                    nc.vector.tensor_copy(z0sb[:], z0p[:])
                    nc.gpsimd.dma_start(cin[:], z0sb[:])
                    nc.gpsimd.collective_compute(
                        "ReduceScatter", AL.add,
                        replica_groups=[list(range(NCORES))],
                        ins=[cin[:].opt()], outs=[cout[:].opt()])

                    # ---- tail: sigmoid(w0 out) -> w1 -> sigmoid -> w2 ----
                    z1row = cpool.tile([BL, H0], F32, tag="z1row")
                    nc.gpsimd.dma_start(z1row[:], cout[:])
                    pz1 = ptail.tile([128, 4 * BL], F32, tag="pz1")
                    z1t = cpool.tile([128, 4 * BL], F32, tag="z1t")
                    for j in range(4):
                        nc.tensor.transpose(pz1[:, j * BL:(j + 1) * BL],
                                            z1row[:, j * 128:(j + 1) * 128],
                                            idn_sb[0:BL, 0:BL])
                        nc.scalar.activation(z1t[:, j * BL:(j + 1) * BL],
                                             pz1[:, j * BL:(j + 1) * BL],
                                             AF.Sigmoid, bias=b0_sb[:, j:j + 1])
                    pz2 = ptail.tile([128, 2 * BL], F32, tag="pz2")
                    z2t = cpool.tile([128, 2 * BL], F32, tag="z2t")
                    for m in range(2):
                        for kj in range(4):
                            nc.tensor.matmul(
                                pz2[:, m * BL:(m + 1) * BL],
                                w1_sb[kj][:, m * 128:(m + 1) * 128],
                                z1t[:, kj * BL:(kj + 1) * BL],
                                start=(kj == 0), stop=(kj == 3))
                        nc.scalar.activation(z2t[:, m * BL:(m + 1) * BL],
                                             pz2[:, m * BL:(m + 1) * BL],
                                             AF.Sigmoid, bias=b1_sb[:, m:m + 1])
                    pz3 = ptail.tile([BL, NCLS], F32, tag="pz3")
                    for m in range(2):
                        nc.tensor.matmul(pz3[:], z2t[:, m * BL:(m + 1) * BL],
                                         w2_sb[m][:], start=(m == 0),
                                         stop=(m == 1))
                    z3 = cpool.tile([BL, NCLS], F32, tag="z3")
                    nc.vector.tensor_tensor(z3[:], pz3[:], b2_sb[:], AL.add)
                    mx = cpool.tile([BL, 1], F32, tag="mx")
                    nc.vector.tensor_reduce(mx[:], z3[:], mybir.AxisListType.X,
                                            AL.max)
                    tm = cpool.tile([BL, NCLS], F32, tag="tm")
                    nc.vector.tensor_scalar(tm[:], z3[:], mx[:, 0:1], None,
                                            AL.subtract)
                    ex = cpool.tile([BL, NCLS], F32, tag="ex")
                    se = cpool.tile([BL, 1], F32, tag="se")
                    nc.scalar.activation(ex[:], tm[:], AF.Exp, accum_out=se[:])
                    ls = cpool.tile([BL, 1], F32, tag="ls")
                    nc.scalar.activation(ls[:], se[:], AF.Ln)
                    osb = cpool.tile([BL, NCLS], F32, tag="osb")
                    nc.vector.tensor_scalar(osb[:], tm[:], ls[:, 0:1], None,
                                            AL.subtract)
                    nc.gpsimd.dma_start(out[:], osb[:])
    return nc


_CACHE = {}


def kernel(x, conv_w, conv_b, w0, b0, w1, b1, w2, b2):
    x = np.ascontiguousarray(np.asarray(x, np.float32))
    conv_w = np.asarray(conv_w, np.float32)
    conv_b = np.asarray(conv_b, np.float32)
    w0 = np.asarray(w0, np.float32)
    w1 = np.ascontiguousarray(np.asarray(w1, np.float32))
    w2 = np.ascontiguousarray(np.asarray(w2, np.float32))
    b0 = np.asarray(b0, np.float32)
    b1 = np.asarray(b1, np.float32)
    b2 = np.asarray(b2, np.float32)

    if "nc" not in _CACHE:
        _CACHE["nc"] = _build()
        _CACHE["perm"] = _w0_perm()
    nc = _CACHE["nc"]
    bf16 = mybir.dt.np(BF16)
    w0p = np.ascontiguousarray(w0[_CACHE["perm"], :]).astype(bf16)

    shared = {
        "cwr": np.ascontiguousarray(
            np.broadcast_to(conv_w.reshape(1, 16), (128, 16))),
        "cbr": np.ascontiguousarray(
            np.broadcast_to(conv_b.reshape(1, OUT_CH), (128, OUT_CH))),
        "tlin": np.linspace(0.0, 1.0, T, dtype=np.float32).reshape(128, 1),
        "ltri": np.ascontiguousarray(
            np.triu(np.ones((128, 128), np.float32), 1)),
        "onef": np.ones((128, 1), np.float32),
        "oner": np.ones((128, 2), np.float32),
        "idn": np.eye(128, dtype=np.float32),
        "w1s": w1, "w2s": w2,
        "b0c": b0.reshape(H0, 1), "b1c": b1.reshape(H1, 1),
        "b2r": np.ascontiguousarray(np.broadcast_to(b2.reshape(1, NCLS),
                                                    (BL, NCLS))),
    }
    in_maps = []
    for c in range(NCORES):
        m = dict(shared)
        m["xs"] = np.ascontiguousarray(
            x[c * BL:(c + 1) * BL, 0].transpose(1, 0, 2))
        m["w0s"] = np.ascontiguousarray(w0p[c * KSH:(c + 1) * KSH, :])
        in_maps.append(m)

    _CACHE["in_maps"] = in_maps
    res = run_bass_kernel_spmd(nc, in_maps, core_ids=list(range(NCORES)))
    return np.concatenate([res.results[c]["out"] for c in range(NCORES)],
                          axis=0)


# revision 33
# speedup vs baseline: 1.4436x; 1.0468x over previous
"""CNN + truncated path-signature (depth 4) + FF head on 8 TRN2 NeuronCores.

Strategy
--------
- Batch data-parallel signature computation: core c handles batches
  [8c, 8c+8) = 32 (batch, out_ch) lanes, T=128 time steps on partitions.
- Signature reformulated to avoid sequential scans:
    dx, P1 (= shifted path), Y (= suffix sums) are free;
    the only prefix scan (level 2, S2pre) is one triangular matmul L @ M2;
    S3 = B^T X;  S4 = B^T R + tmp81'^T (dx(x)dx)/2   (suffix-vector trick),
  so levels 3 and 4 are plain T-contractions on the TensorEngine.
- The 60 MB w0 is row-sharded 8 ways (this is the memory roofline win):
  AllToAll re-shards the signature activations feature-wise, each core
  multiplies its [3690, 512] w0 shard, ReduceScatter returns each core its
  own 8 batches, and the small w1/w2 head finishes per-core.
- w0 rows are permuted host-side to match the kernel's natural feature
  order, so no on-device transposes are needed.
"""
import os
import sys
sys.path.insert(0, "/opt/trn_rl_repo")
if os.environ.get("JAX_PLATFORMS") == "cpu":
    # The SPMD launch needs the axon/neuron PJRT backend.
    os.environ["JAX_PLATFORMS"] = ""

import numpy as np
import bass_rust as _bass_rust
import concourse.bass as bass
import concourse.tile as tile
import concourse.mybir as mybir
from concourse.vector_clock import ScopedClock
from concourse.bass_utils import run_bass_kernel_spmd

F32 = mybir.dt.float32
F32R = mybir.dt.float32r
BF16 = mybir.dt.bfloat16
AL = mybir.AluOpType
AF = mybir.ActivationFunctionType

NCORES = 8
B, T, IN_CH = 64, 128, 32
OUT_CH, CH, D = 4, 8, 9            # conv out-channels, conv width, path dim
BL = B // NCORES                   # local batches = 8
LANES = BL * OUT_CH                # 32 lanes/core
NG = 4                             # lane groups of 8
GL = 8                             # lanes per group
SIGC = 7380                        # per-lane signature channels
KSH = OUT_CH * SIGC // NCORES      # w0 K-shard rows per core = 3690
H0, H1, NCLS = 512, 256, 10


class _SplitDrainTileContext(tile.TileContext):
    """Tile exit drain carries one sem wait per CTRL instruction.

    This container's walrus build rejects >2 sync waits on a CTRL
    instruction; Tile's exit drain waits on the whole global clock.
    Redistribute the waits over nops on the same engine (program order on
    one engine preserves semantics)."""

    MAX_WAITS = 1

    def _split_body_waits(self):
        """Move excess sem waits from any instruction onto preceding nops on
        the same engine (same-engine program order preserves semantics)."""
        nc = self.nc
        for bb in nc.main_func.blocks:
            heavy = [ins for ins in bb.instructions
                     if ins.sync_info and ins.sync_info.on_wait
                     and len(ins.sync_info.on_wait) > self.MAX_WAITS]
            if not heavy:
                continue
            extra = {}
            for ins in heavy:
                w = list(ins.sync_info.on_wait)
                ins.sync_info.on_wait = w[:self.MAX_WAITS]
                nops = []
                for i in range(self.MAX_WAITS, len(w), self.MAX_WAITS):
                    n = nc.engines[ins.engine].nop(hint="wait_split")
                    # pop the freshly appended nop from wherever it landed
                    for bb2 in nc.main_func.blocks:
                        if bb2.instructions and bb2.instructions[-1] is n.ins:
                            bb2.instructions.pop()
                            break
                    for wt in w[i:i + self.MAX_WAITS]:
                        handle = _bass_rust.SemaphoreHandle(wt.ant_name, wt.id)
                        _bass_rust.wait_op(n.ins, handle, wt.wait_value,
                                           "sem-ge", False)
                    nops.append(n.ins)
                extra[id(ins)] = nops
            new_list = []
            for ins in bb.instructions:
                new_list.extend(extra.get(id(ins), ()))
                new_list.append(ins)
            bb.instructions[:] = new_list

    def _drain_and_barrier(self, tick_clock, wait_clock):
        nc = self.nc
        self._split_body_waits()
        probe = nc.sync.nop(hint="tile_exit_wait_0")
        wait_clock.add_sem_waits(
            probe.ins, ScopedClock({None: tick_clock.global_clock})
        )
        waits = list(probe.ins.sync_info.on_wait or [])
        probe.ins.sync_info.on_wait = waits[:1]
        for w in waits[1:]:
            n = nc.sync.nop(hint="tile_exit_wait")
            handle = _bass_rust.SemaphoreHandle(w.ant_name, w.id)
            _bass_rust.wait_op(n.ins, handle, w.wait_value, "sem-ge", False)
        nc.sync.drain()
        nc.all_engine_barrier()
        assert self.sems is not None
        popped = nc._tile_sem_poison_stack.pop()
        assert popped is self._sem_poison
        nc.clear_and_free_semaphores(list(self.sems.allocated().values()))
        nc.all_engine_barrier()


def _ap(t, extra, *dims):
    """AP over tile t's buffer: partition dim from the tile, custom free dims.

    dims[0] may override the partition [step, count]."""
    base = t[:]
    return bass.AP(base.tensor, base.offset + extra, list(dims))


def _w0_perm():
    """Row permutation p s.t. w0_permuted[i] = w0[p[i]] matches the kernel's
    feature order: per oc: [S1(9) | S2(81)] then rows 90 + ij*90 + c with
    c<81 -> level4 (ij,kl=c), c>=81 -> level3 (ij, k=c-81)."""
    p = np.empty(OUT_CH * SIGC, np.int64)
    i = 0
    for oc in range(OUT_CH):
        base = oc * SIGC
        p[i:i + 90] = base + np.arange(90)          # S1 then S2, native order
        i += 90
        for ij in range(81):
            # level-4 block (ij, kl) for kl in 0..80 -> orig 819 + ij*81 + kl
            p[i:i + 81] = base + 819 + ij * 81 + np.arange(81)
            i += 81
            # level-3 (ij, k) for k in 0..8 -> orig 90 + ij*9 + k
            p[i:i + 9] = base + 90 + ij * 9 + np.arange(9)
            i += 9
    assert i == OUT_CH * SIGC
    return p


def _build():
    nc = bass.Bass(num_devices=NCORES, target_bir_lowering=True, trn_type="TRN2")

    # ---- per-core DRAM inputs ----
    xs = nc.dram_tensor("xs", [T, BL, IN_CH], F32, kind="ExternalInput")
    cwr = nc.dram_tensor("cwr", [128, 16], F32, kind="ExternalInput")
    cbr = nc.dram_tensor("cbr", [128, OUT_CH], F32, kind="ExternalInput")
    tlin = nc.dram_tensor("tlin", [128, 1], F32, kind="ExternalInput")
    ltri = nc.dram_tensor("ltri", [128, 128], F32R, kind="ExternalInput")
    onef = nc.dram_tensor("onef", [128, 1], F32, kind="ExternalInput")
    oner = nc.dram_tensor("oner", [128, 2], F32R, kind="ExternalInput")
    idn = nc.dram_tensor("idn", [128, 128], F32, kind="ExternalInput")
    w0s = nc.dram_tensor("w0s", [KSH, H0], BF16, kind="ExternalInput")
    w1s = nc.dram_tensor("w1s", [H0, H1], F32, kind="ExternalInput")
    w2s = nc.dram_tensor("w2s", [H1, NCLS], F32, kind="ExternalInput")
    b0c = nc.dram_tensor("b0c", [H0, 1], F32, kind="ExternalInput")
    b1c = nc.dram_tensor("b1c", [H1, 1], F32, kind="ExternalInput")
    b2r = nc.dram_tensor("b2r", [BL, NCLS], F32, kind="ExternalInput")
    out = nc.dram_tensor("out", [BL, NCLS], F32, kind="ExternalOutput")

    with _SplitDrainTileContext(nc) as tc:
        with tc.tile_pool(name="dram", bufs=1, space="DRAM") as dram:
            zl = dram.tile([NCORES * KSH, BL], BF16)     # local z^T, f'-order
            zex = dram.tile([NCORES * KSH, BL], BF16)    # post-A2A shard
            cin = dram.tile([B, H0], F32)
            cout = dram.tile([BL, H0], F32)
            prow = dram.tile([1, LANES * D], F32)        # p[T-1] bounce

            with tc.tile_pool(name="const", bufs=1) as cpool, \
                 tc.tile_pool(name="w0p", bufs=1) as w0pool, \
                 tc.tile_pool(name="prep", bufs=1) as ppool:
                xs_sb = cpool.tile([128, BL * IN_CH], F32)
                nc.scalar.dma_start(xs_sb[:], _ap(xs, 0, [BL * IN_CH, 128],
                                                [1, BL * IN_CH]))
                cw_sb = cpool.tile([128, 16], F32)
                nc.scalar.dma_start(cw_sb[:], cwr[:])
                cb_sb = cpool.tile([128, OUT_CH], F32)
                nc.scalar.dma_start(cb_sb[:], cbr[:])
                tl_sb = cpool.tile([128, 1], F32)
                nc.scalar.dma_start(tl_sb[:], tlin[:])
                lt_sb = cpool.tile([128, 128], F32R)
                nc.scalar.dma_start(lt_sb[:], ltri[:])
                onef_sb = cpool.tile([128, 1], F32)
                nc.scalar.dma_start(onef_sb[:], onef[:])
                oner_sb = cpool.tile([128, 2], F32R)
                nc.scalar.dma_start(oner_sb[:], oner[:])
                idn_sb = cpool.tile([128, 128], F32)
                nc.scalar.dma_start(idn_sb[:], idn[:])
                w1_sb = [cpool.tile([128, H1], F32, tag=f"w1_{j}", name=f"w1_{j}")
                         for j in range(4)]
                for j in range(4):
                    nc.scalar.dma_start(w1_sb[j][:], w1s[j * 128:(j + 1) * 128, :])
                w2_sb = [cpool.tile([128, NCLS], F32, tag=f"w2_{j}", name=f"w2_{j}")
                         for j in range(2)]
                for j in range(2):
                    nc.scalar.dma_start(w2_sb[j][:], w2s[j * 128:(j + 1) * 128, :])
                b0_sb = cpool.tile([128, 4], F32)
                nc.scalar.dma_start(b0_sb[:], _ap(b0c, 0, [1, 128], [128, 4]))
                b1_sb = cpool.tile([128, 2], F32)
                nc.scalar.dma_start(b1_sb[:], _ap(b1c, 0, [1, 128], [128, 2]))
                b2_sb = cpool.tile([BL, NCLS], F32)
                nc.scalar.dma_start(b2_sb[:], b2r[:])
                # ---- prep: conv -> path p, then dx, P1, Y, u, u2, A ----
                W = LANES * D  # 288
                p = ppool.tile([128, W], F32, tag="p")
                dx = ppool.tile([128, W], F32, tag="dx")
                p1 = ppool.tile([128, W], F32, tag="p1")
                yt = ppool.tile([128, W], F32, tag="yt")
                pl = ppool.tile([128, W], F32, tag="pl")
                ut = ppool.tile([128, W], F32, tag="ut")
                u2 = ppool.tile([128, W], F32, tag="u2")
                at = ppool.tile([128, W], F32, tag="at")

                # conv, batched over (oc, b, ch): p_data = sum_k x_k * w[oc,k]
                pdst = _ap(p, 1, [W, 128], [D, OUT_CH], [4 * D, BL], [1, CH])
                tmpc = ppool.tile([128, BL * IN_CH], F32, tag="tmpc")
                tview = _ap(tmpc, 0, [BL * IN_CH, 128],
                            [CH * BL, OUT_CH], [CH, BL], [1, CH])
                for k in range(4):
                    xsv = _ap(xs_sb, k, [BL * IN_CH, 128],
                              [0, OUT_CH], [IN_CH, BL], [4, CH])
                    cwv = _ap(cw_sb, k, [16, 128], [4, OUT_CH], [0, BL], [0, CH])
                    if k == 0:
                        nc.vector.tensor_tensor(pdst, xsv, cwv, AL.mult)
                    else:
                        nc.vector.tensor_tensor(tview, xsv, cwv, AL.mult)
                        nc.vector.tensor_tensor(pdst, pdst, tview, AL.add)
                cbv = _ap(cb_sb, 0, [OUT_CH, 128], [1, OUT_CH], [0, BL], [0, CH])
                nc.vector.tensor_tensor(pdst, pdst, cbv, AL.add)
                # time channel into col 0 of every lane
                nc.vector.tensor_copy(_ap(p, 0, [W, 128], [D, LANES]),
                                      _ap(tl_sb, 0, [1, 128], [0, LANES]))
                # P1 = p shifted down one step (DMA: compute engines cannot
                # address unaligned partition bases), then dx = p - P1.
                nc.gpsimd.memset(p1[0:1, :], 0.0)
                nc.gpsimd.dma_start(p1[1:128, :], p[0:127, :])
                nc.vector.tensor_tensor(dx[:], p[:], p1[:], AL.subtract)
                # Y[t] = p[T-1] - p[t]  (broadcast last row via DRAM bounce)
                nc.scalar.dma_start(prow[:], p[127:128, :])
                nc.scalar.dma_start(pl[:], _ap(prow, 0, [0, 128], [1, W]))
                # S1 = p[T-1] per lane: gather straight out of the bounce row
                z1tmp = ppool.tile([9, LANES], F32, tag="z1tmp")
                nc.scalar.dma_start(
                    z1tmp[:], _ap(prow, 0, [1, 9], [D, LANES]))
                nc.vector.tensor_tensor(yt[:], pl[:], p[:], AL.subtract)
                # ut = P1/2 + dx/6, u2 = P1/3 + dx/12, A = P1 + dx/2,
                # dxh = dx/2 (scalars pre-folded so the group outer products
                # are plain TensorTensor ops)
                nc.vector.tensor_scalar(ut[:], p1[:], 0.5, None, AL.mult)
                nc.vector.scalar_tensor_tensor(ut[:], dx[:], 1.0 / 6, ut[:],
                                               AL.mult, AL.add)
                nc.vector.tensor_scalar(u2[:], p1[:], 1.0 / 3, None, AL.mult)
                nc.vector.scalar_tensor_tensor(u2[:], dx[:], 1.0 / 12, u2[:],
                                               AL.mult, AL.add)
                nc.vector.scalar_tensor_tensor(at[:], dx[:], 0.5, p1[:],
                                               AL.mult, AL.add)
                dxh = ppool.tile([128, W], F32, tag="dxh")
                nc.vector.tensor_scalar(dxh[:], dx[:], 0.5, None, AL.mult)
                # preload ACT function tables off the critical tail
                dum = ppool.tile([1, 4], F32, tag="dum")
                nc.scalar.activation(dum[0:1, 0:1], tl_sb[0:1, 0:1], AF.Sigmoid)
                nc.scalar.activation(dum[0:1, 1:2], tl_sb[0:1, 0:1], AF.Exp)
                nc.scalar.activation(dum[0:1, 2:3], onef_sb[0:1, 0:1], AF.Ln)

                # ---- per-lane feature tiles, filled group by group ----
                zt4all = cpool.tile([81, OUT_CH * 720], BF16, tag="zt4all",
                                    name="zt4all")
                z1all = cpool.tile([9, LANES], BF16, tag="z1all", name="z1all")
                z2all = cpool.tile([81, LANES], BF16, tag="z2all", name="z2all")

                with tc.tile_pool(name="grp", bufs=2) as gpool, \
                     tc.tile_pool(name="ps2", bufs=2, space="PSUM") as ps2, \
                     tc.tile_pool(name="ptab", bufs=1, space="PSUM") as ptab, \
                     tc.tile_pool(name="ps12", bufs=1, space="PSUM") as ps12:
                    for g in range(NG):
                        off = g * GL * D  # col offset into the 288-wide tiles
                        GW = GL * 81      # 648

                        def o_ij(t, st=1):  # [lane, i(step), j(bcast)] view
                            return _ap(t, off, [W, 128], [D, GL], [st, D], [0, D])

                        def o_ji(t, st=1):  # [lane, i(bcast), j(step)] view
                            return _ap(t, off, [W, 128], [D, GL], [0, D], [st, D])

                        m2 = gpool.tile([128, GW], F32R, tag="m2")
                        m2v = _ap(m2, 0, [GW, 128], [81, GL], [D, D], [1, D])
                        nc.vector.tensor_tensor(m2v, o_ij(at), o_ji(dx), AL.mult)


                        s2 = ps2.tile([128, GW], F32, tag="s2")
                        nc.tensor.matmul(s2[:, 0:512], lt_sb[:], m2[:, 0:512],
                                         start=True, stop=True)
                        nc.tensor.matmul(s2[:, 512:GW], lt_sb[:], m2[:, 512:GW],
                                         start=True, stop=True)

                        bt = gpool.tile([128, GW], F32R, tag="bt")
                        btv = _ap(bt, 0, [GW, 128], [81, GL], [D, D], [1, D])
                        nc.vector.tensor_tensor(btv, o_ij(ut), o_ji(dx), AL.mult)
                        nc.vector.tensor_tensor(bt[:], bt[:], s2[:], AL.add)

                        t8 = gpool.tile([128, GW], F32R, tag="t8")
                        t8v = _ap(t8, 0, [GW, 128], [81, GL], [D, D], [1, D])
                        nc.vector.tensor_tensor(t8v, o_ij(u2), o_ji(dx), AL.mult)
                        nc.vector.tensor_tensor(t8[:], t8[:], s2[:], AL.add)

                        # 82-wide lane stride: f32r matmuls need even N,
                        # so the mmTB rhs is [128, 82] with a zero last column
                        q2 = gpool.tile([128, GL * 82], F32R, tag="q2")
                        q2v = _ap(q2, 0, [GL * 82, 128], [82, GL], [D, D], [1, D])
                        nc.gpsimd.tensor_tensor(q2v, o_ij(dxh), o_ji(dx), AL.mult)
                        nc.gpsimd.tensor_scalar(
                            _ap(q2, 81, [GL * 82, 128], [82, GL]),
                            _ap(dxh, 0, [LANES * D, 128], [0, GL]),
                            0.0, None, AL.mult)

                        rx = gpool.tile([128, GL * 90], F32R, tag="rx")
                        rxv = _ap(rx, 0, [GL * 90, 128], [90, GL], [D, D], [1, D])
                        nc.gpsimd.tensor_tensor(rxv, o_ij(dx), o_ji(yt), AL.mult)
                        nc.vector.tensor_copy(
                            _ap(rx, 81, [GL * 90, 128], [90, GL], [1, D]),
                            _ap(dx, off, [W, 128], [D, GL], [1, D]))

                        tab = ptab.tile([128, 1024], F32, tag="tab")
                        s12 = ps12.tile([128, 2 * GL], F32, tag="s12")
                        for l in range(GL):
                            lane = g * GL + l
                            bloc, oc = lane // OUT_CH, lane % OUT_CH
                            nc.tensor.matmul(
                                _ap(tab, 128 * l, [1024, 81], [1, 90]),
                                bt[:, l * 81:(l + 1) * 81],
                                rx[:, l * 90:(l + 1) * 90],
                                start=True, stop=False)
                            nc.tensor.matmul(
                                _ap(tab, 128 * l, [1024, 81], [1, 82]),
                                t8[:, l * 81:(l + 1) * 81],
                                q2[:, l * 82:(l + 1) * 82],
                                start=False, stop=True)
                            nc.tensor.matmul(s12[0:81, 2 * l:2 * l + 2],
                                             m2[:, l * 81:(l + 1) * 81],
                                             oner_sb[:], start=True, stop=True)
                        # batched PSUM->SBUF evacuation for the whole group:
                        # lanes l = bo*4 + oc, local batch bloc = 2g + bo
                        nc.vector.tensor_copy(
                            _ap(zt4all, 2 * g, [2880, 81],
                                [720, OUT_CH], [8, 90]),
                            _ap(tab, 0, [1024, 81],
                                [128, OUT_CH], [1, 90]))
                        nc.scalar.activation(
                            _ap(zt4all, 2 * g + 1, [2880, 81],
                                [720, OUT_CH], [8, 90]),
                            _ap(tab, 512, [1024, 81],
                                [128, OUT_CH], [1, 90]),
                            AF.Copy)
                        nc.scalar.activation(
                            _ap(z2all, 2 * g, [LANES, 81], [1, 2], [8, OUT_CH]),
                            _ap(s12, 0, [2 * GL, 81], [8, 2], [2, OUT_CH]),
                            AF.Copy)

                # ---- pack z^T shard-ordered into DRAM and exchange ----
                nc.scalar.activation(
                    _ap(z1all, 0, [LANES, 9], [8, OUT_CH], [1, BL]),
                    _ap(z1tmp, 0, [LANES, 9], [1, OUT_CH], [OUT_CH, BL]),
                    AF.Copy)
                for oc in range(OUT_CH):
                    base = oc * SIGC * BL
                    eng = nc.scalar if oc < 2 else nc.sync
                    eng.dma_start(
                        _ap(zl, base, [8, 9], [1, 8]),
                        z1all[:, oc * BL:(oc + 1) * BL])
                    eng.dma_start(
                        _ap(zl, base + 9 * BL, [8, 81], [1, 8]),
                        z2all[:, oc * BL:(oc + 1) * BL])
                    eng.dma_start(
                        _ap(zl, base + 90 * BL, [720, 81], [8, 90], [1, 8]),
                        zt4all[:, oc * 720:(oc + 1) * 720])
                # w0 shard prefetch (emitted here so these no-dep DMAs sit
                # behind nothing critical on the sync queue, and stream
                # during the signature phase / AllToAll)
                w0_t = []
                for i in range(29):
                    k0 = i * 128
                    kn = min(128, KSH - k0)
                    wt = w0pool.tile([128, H0], BF16, tag=f"w0_{i}", name=f"w0_{i}")
                    nc.sync.dma_start(wt[0:kn, :], w0s[k0:k0 + kn, :])
                    w0_t.append((wt, k0, kn))
                nc.gpsimd.collective_compute(
                    "AllToAll", AL.bypass,
                    replica_groups=[list(range(NCORES))],
                    ins=[zl[:].opt()], outs=[zex[:].opt()])

                # ---- z0 = z @ w0 partial over this core's K shard ----
                with tc.tile_pool(name="zex", bufs=29) as zpool, \
                     tc.tile_pool(name="pz0", bufs=1, space="PSUM") as pz0p, \
                     tc.tile_pool(name="ptail", bufs=1, space="PSUM") as ptail:
                    z0p = pz0p.tile([B, H0], F32, tag="z0p")
                    for i, (wt, k0, kn) in enumerate(w0_t):
                        ztl = zpool.tile([128, B], BF16, tag="ztl")
                        eng = (nc.sync, nc.scalar, nc.gpsimd)[i % 3]
                        eng.dma_start(
                            ztl[0:kn, :],
                            _ap(zex, k0 * BL, [BL, kn],
                                [KSH * BL, NCORES], [1, BL]))
                        nc.tensor.matmul(z0p[:], ztl[0:kn, :], wt[0:kn, :],
                                         start=(i == 0), stop=(i == 28))
                    z0sb = cpool.tile([B, H0], F32, tag="z0sb", name="z0sb")
                    nc.vector.tensor_copy(z0sb[:], z0p[:])
                    nc.gpsimd.dma_start(cin[:], z0sb[:])
                    nc.gpsimd.collective_compute(
                        "ReduceScatter", AL.add,
                        replica_groups=[list(range(NCORES))],
                        ins=[cin[:].opt()], outs=[cout[:].opt()])

                    # ---- tail: sigmoid(w0 out) -> w1 -> sigmoid -> w2 ----
                    z1row = cpool.tile([BL, H0], F32, tag="z1row")
                    nc.gpsimd.dma_start(z1row[:], cout[:])
                    pz1 = ptail.tile([128, 4 * BL], F32, tag="pz1")
                    z1t = cpool.tile([128, 4 * BL], F32, tag="z1t")
                    for j in range(4):
                        nc.tensor.transpose(pz1[:, j * BL:(j + 1) * BL],
                                            z1row[:, j * 128:(j + 1) * 128],
                                            idn_sb[0:BL, 0:BL])
                        nc.scalar.activation(z1t[:, j * BL:(j + 1) * BL],
                                             pz1[:, j * BL:(j + 1) * BL],
                                             AF.Sigmoid, bias=b0_sb[:, j:j + 1])
                    pz2 = ptail.tile([128, 2 * BL], F32, tag="pz2")
                    z2t = cpool.tile([128, 2 * BL], F32, tag="z2t")
                    for m in range(2):
                        for kj in range(4):
                            nc.tensor.matmul(
                                pz2[:, m * BL:(m + 1) * BL],
                                w1_sb[kj][:, m * 128:(m + 1) * 128],
                                z1t[:, kj * BL:(kj + 1) * BL],
                                start=(kj == 0), stop=(kj == 3))
                        nc.scalar.activation(z2t[:, m * BL:(m + 1) * BL],
                                             pz2[:, m * BL:(m + 1) * BL],
                                             AF.Sigmoid, bias=b1_sb[:, m:m + 1])
                    pz3 = ptail.tile([BL, NCLS], F32, tag="pz3")
                    for m in range(2):
                        nc.tensor.matmul(pz3[:], z2t[:, m * BL:(m + 1) * BL],
                                         w2_sb[m][:], start=(m == 0),
                                         stop=(m == 1))
                    z3 = cpool.tile([BL, NCLS], F32, tag="z3")
                    nc.vector.tensor_tensor(z3[:], pz3[:], b2_sb[:], AL.add)
                    mx = cpool.tile([BL, 1], F32, tag="mx")
                    nc.vector.tensor_reduce(mx[:], z3[:], mybir.AxisListType.X,
                                            AL.max)
                    tm = cpool.tile([BL, NCLS], F32, tag="tm")
                    nc.vector.tensor_scalar(tm[:], z3[:], mx[:, 0:1], None,
                                            AL.subtract)
                    ex = cpool.tile([BL, NCLS], F32, tag="ex")
                    se = cpool.tile([BL, 1], F32, tag="se")
                    nc.scalar.activation(ex[:], tm[:], AF.Exp, accum_out=se[:])
                    ls = cpool.tile([BL, 1], F32, tag="ls")
                    nc.scalar.activation(ls[:], se[:], AF.Ln)
                    osb = cpool.tile([BL, NCLS], F32, tag="osb")
                    nc.vector.tensor_scalar(osb[:], tm[:], ls[:, 0:1], None,
                                            AL.subtract)
                    nc.gpsimd.dma_start(out[:], osb[:])
    return nc


_CACHE = {}


def kernel(x, conv_w, conv_b, w0, b0, w1, b1, w2, b2):
    x = np.ascontiguousarray(np.asarray(x, np.float32))
    conv_w = np.asarray(conv_w, np.float32)
    conv_b = np.asarray(conv_b, np.float32)
    w0 = np.asarray(w0, np.float32)
    w1 = np.ascontiguousarray(np.asarray(w1, np.float32))
    w2 = np.ascontiguousarray(np.asarray(w2, np.float32))
    b0 = np.asarray(b0, np.float32)
    b1 = np.asarray(b1, np.float32)
    b2 = np.asarray(b2, np.float32)

    if "nc" not in _CACHE:
        _CACHE["nc"] = _build()
        _CACHE["perm"] = _w0_perm()
    nc = _CACHE["nc"]
    bf16 = mybir.dt.np(BF16)
    w0p = np.ascontiguousarray(w0[_CACHE["perm"], :]).astype(bf16)

    shared = {
        "cwr": np.ascontiguousarray(
            np.broadcast_to(conv_w.reshape(1, 16), (128, 16))),
        "cbr": np.ascontiguousarray(
            np.broadcast_to(conv_b.reshape(1, OUT_CH), (128, OUT_CH))),
        "tlin": np.linspace(0.0, 1.0, T, dtype=np.float32).reshape(128, 1),
        "ltri": np.ascontiguousarray(
            np.triu(np.ones((128, 128), np.float32), 1)),
        "onef": np.ones((128, 1), np.float32),
        "oner": np.ones((128, 2), np.float32),
        "idn": np.eye(128, dtype=np.float32),
        "w1s": w1, "w2s": w2,
        "b0c": b0.reshape(H0, 1), "b1c": b1.reshape(H1, 1),
        "b2r": np.ascontiguousarray(np.broadcast_to(b2.reshape(1, NCLS),
                                                    (BL, NCLS))),
    }
    in_maps = []
    for c in range(NCORES):
        m = dict(shared)
        m["xs"] = np.ascontiguousarray(
            x[c * BL:(c + 1) * BL, 0].transpose(1, 0, 2))
        m["w0s"] = np.ascontiguousarray(w0p[c * KSH:(c + 1) * KSH, :])
        in_maps.append(m)

    _CACHE["in_maps"] = in_maps
    res = run_bass_kernel_spmd(nc, in_maps, core_ids=list(range(NCORES)))
    return np.concatenate([res.results[c]["out"] for c in range(NCORES)],
                          axis=0)
